# revision 2
# baseline (speedup 1.0000x reference)
"""EfficientViT attention block on 8 TRN2 NeuronCores.

Sharding: 8 cores = 4 images x 2 row-halves (64 rows each + halos); s=1 cores
get vertically flipped images + dy-flipped conv weights (identical SPMD
program). kv partial sums AllReduced pairwise ([128,288] f32).

v2 vs baseline: all intermediates SBUF-resident (no ms/att/attf/h1 DRAM
round-trips), dw5+grouped-pw fused into block-diagonal PE matmuls for tiles
0-1 (DVE for tile 2), per-row DMA transposes replaced by PE transposes + Pool
evacuation, attention+projection fused per 4-row chunk, dw3 split across
PE/DVE/Pool, bf16 output (host casts to f32).
"""
import contextlib

import numpy as np
import ml_dtypes

import concourse.bass as bass
import concourse.bacc as bacc
import concourse.tile as tile
from concourse import mybir
from concourse import bass_utils

F32 = mybir.dt.float32
BF16 = mybir.dt.bfloat16
Alu = mybir.AluOpType
AF = mybir.ActivationFunctionType
BF = ml_dtypes.bfloat16

BN_EPS = 1e-5
NCORES = 8

_CACHE = {}
DBG = False

ATT_SRCS = {0: (0, 1, 2), 1: (2, 3, 4), 2: (4, 5)}  # att9 tile a -> multi tiles

# ---- engine split knobs ----
# dw5: t0 + t1-rows<24 fused on PE; t1-rows>=24 + t2-rows<36 on DVE;
# t2-rows>=36 on Pool (see _head emission)
DW3_ENG = ("pe", "pe", "pe", "pe", "dve", "pe")  # per mid-tile t


def q_chan(g, e):
    return 24 * g + e if g < 16 else 384 + 24 * (g - 16) + e


def _chunks(lo, hi, step):
    out = []
    r = lo
    while r < hi:
        n = min(step, hi - r)
        out.append((r, n))
        r += n
    return out


def build_program():
    nc = bacc.Bacc("TRN2", target_bir_lowering=False, debug=False,
                   enable_asserts=False, num_devices=NCORES)
    d = {}
    def din(name, shape, dt):
        d[name] = nc.dram_tensor(name, shape, dt, kind="ExternalInput").ap()
    din("xr", [128, 72, 130], BF16)
    din("xo", [128, 72, 130], BF16)
    din("wc", [128, 27 * 128], BF16)
    din("cb", [128, 3], F32)
    din("cm", [128, 3], F32)
    din("wf", [128, 3 * 25 * 128], BF16)
    din("dw5", [128, 75], F32)
    din("pww", [128, 3 * 128], BF16)
    din("pjw", [128, 3 * 128], BF16)
    din("pjb", [128, 1], F32)
    din("m1w", [128, 6 * 128], BF16)
    din("m1b", [128, 6], F32)
    din("h1b", [128, 6], F32)
    din("dw3", [128, 54], F32)
    din("dw3d", [128, 54 * 128], BF16)
    din("m2b", [128, 6], F32)
    din("h2b", [128, 6], F32)
    din("m3w", [128, 6 * 128], BF16)
    din("m3b", [128, 1], F32)
    din("rb", [128, 66, 128], BF16)
    din("idn", [128, 128], BF16)
    din("eb", [12, 96], BF16)
    d["out"] = nc.dram_tensor("out", [128, 64, 128], BF16,
                              kind="ExternalOutput").ap()
    if DBG:
        for nm, sh, dt in (("dqkv", [3, 128, 70, 132], BF16),
                           ("dms", [3, 128, 66, 128], BF16),
                           ("dcomp", [128, 288], F32),
                           ("dcompR", [128, 288], F32),
                           ("dattf", [128, 66 * 128], BF16),
                           ("dh1", [6, 128, 66, 130], BF16)):
            d[nm] = nc.dram_tensor(nm, sh, dt, kind="ExternalOutput").ap()
    with tile.TileContext(nc) as tc:
        _emit(nc, tc, d)
    nc.compile()
    return nc


def _emit(nc, tc, d):
    env = {}
    with contextlib.ExitStack() as ctx:
        wp = ctx.enter_context(tc.tile_pool(name="wp", bufs=1))
        dram = ctx.enter_context(tc.tile_pool(name="dram", bufs=1, space="DRAM"))
        env["wp"] = wp

        def wtile(name, shape, dt, pool=wp):
            t = pool.tile(shape, dt, tag=name, name=name)
            nc.sync.dma_start(out=t, in_=d[name])
            return t
        env["wtile"] = wtile

        for nm, sh, dt in (("pjw", [128, 3 * 128], BF16), ("pjb", [128, 1], F32),
                           ("m1w", [128, 6 * 128], BF16), ("m1b", [128, 6], F32),
                           ("h1b", [128, 6], F32), ("dw3", [128, 54], F32),
                           ("m2b", [128, 6], F32), ("h2b", [128, 6], F32),
                           ("m3w", [128, 6 * 128], BF16), ("m3b", [128, 1], F32)):
            env[nm] = wtile(nm, sh, dt)
        env["attf"] = wp.tile([128, 66 * 128], BF16, tag="attf", name="attf")
        env["half"] = wp.tile([128, 1], F32, tag="half", name="half")
        nc.vector.memset(env["half"], 0.5)
        env["epsv"] = wp.tile([128, 1], F32, tag="epsv", name="epsv")
        nc.vector.memset(env["epsv"], 1e-6)
        env["comp"] = wp.tile([128, 288], F32, tag="comp", name="comp")
        env["compR"] = wp.tile([128, 288], F32, tag="compR", name="compR")
        env["cc_in"] = dram.tile([128, 288], F32, tag="cc_in", name="cc_in")
        env["cc_out"] = dram.tile([128, 288], F32, tag="cc_out", name="cc_out")
        env["rscr"] = dram.tile([12, 1536], BF16, tag="rscr", name="rscr")

        with tc.tile_pool(name="qkvms", bufs=1) as qp:
            env["qkv"] = [qp.tile([128, 70, 132], BF16, tag=f"qkv{t}",
                                  name=f"qkv{t}") for t in range(3)]
            env["ms"] = [qp.tile([128, 66, 128], BF16, tag=f"ms{t}",
                                 name=f"ms{t}") for t in range(3)]
            with contextlib.ExitStack() as hctx:
                _head(nc, tc, d, env, hctx)
            with contextlib.ExitStack() as mctx:
                _mid(nc, tc, d, env, mctx)
        with tc.tile_pool(name="tailp", bufs=1) as tp:
            env["h1"] = [tp.tile([128, 66, 130], BF16, tag=f"h1_{t}",
                                 name=f"h1_{t}") for t in range(6)]
            env["dw3d"] = wtile("dw3d", [128, 54 * 128], BF16, pool=tp)
            with contextlib.ExitStack() as tctx:
                _tail(nc, tc, d, env, tctx)


def _head(nc, tc, d, env, ctx):
    """conv qkv + dw5/pw (ms) + per-row transposes + kv accumulation."""
    qkv, ms = env["qkv"], env["ms"]
    wtile = env["wtile"]

    xp = ctx.enter_context(tc.tile_pool(name="xp", bufs=1))
    wc = wtile("wc", [128, 27 * 128], BF16, pool=xp)
    xr = xp.tile([128, 72, 130], BF16, tag="xr", name="xr")
    xo = xp.tile([128, 72, 130], BF16, tag="xo", name="xo")
    for (r0, rn) in ((0, 28), (28, 22), (50, 22)):
        nc.sync.dma_start(out=xr[:, r0:r0 + rn, :], in_=d["xr"][:, r0:r0 + rn, :])
        nc.sync.dma_start(out=xo[:, r0:r0 + rn, :], in_=d["xo"][:, r0:r0 + rn, :])
    wf = wtile("wf", [128, 3 * 25 * 128], BF16, pool=xp)
    cb = wtile("cb", [128, 3], F32, pool=xp)
    cm = wtile("cm", [128, 3], F32, pool=xp)
    dw5 = wtile("dw5", [128, 75], F32, pool=xp)
    pww = wtile("pww", [128, 3 * 128], BF16, pool=xp)
    idn = wtile("idn", [128, 128], BF16, pool=xp)
    ones = xp.tile([128, 1], BF16, tag="ones", name="ones")
    nc.vector.memset(ones, 1.0)
    env["cm"] = cm

    # pad memsets (cols 0:2 / 130:132 and top rows 0:3 of each qkv tile)
    for t in range(3):
        nc.vector.memset(qkv[t][:, :, 0:2], 0.0)
        nc.vector.memset(qkv[t][:, :, 130:132], 0.0)
        nc.vector.memset(qkv[t][:, 0:3, :], 0.0)

    cps = ctx.enter_context(tc.tile_pool(name="cps", bufs=2, space="PSUM"))
    fps = ctx.enter_context(tc.tile_pool(name="fps", bufs=2, space="PSUM"))
    tps = ctx.enter_context(tc.tile_pool(name="tps", bufs=3, space="PSUM"))
    kps = ctx.enter_context(tc.tile_pool(name="kps", bufs=1, space="PSUM"))
    mtp = ctx.enter_context(tc.tile_pool(name="mtp", bufs=3))
    dtp = ctx.enter_context(tc.tile_pool(name="dtp", bufs=1))
    app = ctx.enter_context(tc.tile_pool(name="app", bufs=1))

    kvps = kps.tile([128, 288], F32, tag="kvps", name="kvps")

    conv_chunks = _chunks(3, 70, 4)            # qkv tile rows (17)
    dw_chunks = _chunks(0, 66, 4)              # ms rows (17)

    def emit_conv(t, chunks):
        src = xr if t == 0 else xo
        for (r0, rn) in chunks:
            ps = cps.tile([128, 512], F32, tag="cps", name="cps")
            psv = ps[:, 0:rn * 128].rearrange("p (r w) -> p r w", w=128)
            first = True
            for dy in range(3):
                for dx in range(3):
                    k = (t * 9 + dy * 3 + dx) * 128
                    nc.tensor.matmul(
                        psv, wc[:, k:k + 128],
                        src[:, r0 + dy:r0 + dy + rn, dx:dx + 128],
                        start=first, stop=(dy == 2 and dx == 2))
                    first = False
            nc.scalar.activation(
                out=qkv[t][:, r0:r0 + rn, 2:130], in_=psv,
                func=AF.Identity, bias=cb[:, t:t + 1], scale=1.0)

    def emit_dwf(t, chunks):
        # fused dw5x5 + grouped-pw on PE: 25 block-diag matmuls per chunk
        for (c0, cn) in chunks:
            ps = fps.tile([128, 512], F32, tag="fps", name="fps")
            psv = ps[:, 0:cn * 128].rearrange("p (r w) -> p r w", w=128)
            for tap in range(25):
                dy, dx = tap // 5, tap % 5
                k = (t * 25 + tap) * 128
                nc.tensor.matmul(
                    psv, wf[:, k:k + 128],
                    qkv[t][:, c0 + dy:c0 + dy + cn, dx:dx + 128],
                    start=(tap == 0), stop=(tap == 24))
            nc.scalar.activation(out=ms[t][:, c0:c0 + cn, :], in_=psv,
                                 func=AF.Copy)
            nc.vector.tensor_scalar_max(
                out=ms[t][:, c0:c0 + cn, :], in0=ms[t][:, c0:c0 + cn, :],
                scalar1=cm[:, t:t + 1])

    def emit_dw5_dve(t, o0, o1):
        # dw5 via TS-mul + TT-add on DVE, acc in-place in ms[t]
        for (r0, n) in _chunks(o0, o1, 22):
            acc = ms[t][:, r0:r0 + n, :]
            for tap in range(25):
                dy, dx = tap // 5, tap % 5
                w_ap = dw5[:, t * 25 + tap:t * 25 + tap + 1]
                win = qkv[t][:, r0 + dy:r0 + dy + n, dx:dx + 128]
                if tap == 0:
                    nc.vector.tensor_scalar_mul(out=acc, in0=win, scalar1=w_ap)
                else:
                    tmp = dtp.tile([128, 22, 128], BF16, tag="dwtmp",
                                   name="dwtmp")
                    nc.vector.tensor_scalar_mul(out=tmp[:, 0:n, :], in0=win,
                                                scalar1=w_ap)
                    nc.vector.tensor_add(out=acc, in0=acc, in1=tmp[:, 0:n, :])

    def emit_dw5_ap(t, o0, o1, taps):
        # dw5 with multiplies on DVE (cheap 4x mode) and adds on Pool
        for (r0, n) in _chunks(o0, o1, 22):
            acc = ms[t][:, r0:r0 + n, :]
            for tap in taps:
                dy, dx = tap // 5, tap % 5
                w_ap = dw5[:, t * 25 + tap:t * 25 + tap + 1]
                win = qkv[t][:, r0 + dy:r0 + dy + n, dx:dx + 128]
                if tap == 0:
                    nc.vector.tensor_scalar_mul(out=acc, in0=win, scalar1=w_ap)
                else:
                    tmp = app.tile([128, 22, 128], BF16, tag="aptmp",
                                   name="aptmp")
                    nc.vector.tensor_scalar_mul(out=tmp[:, 0:n, :], in0=win,
                                                scalar1=w_ap)
                    nc.gpsimd.tensor_add(out=acc, in0=acc, in1=tmp[:, 0:n, :])

    def emit_pw(t, chunks):
        # grouped pw over raw dw5 acc already in ms[t]; evac back in place
        for (c0, cn) in chunks:
            ps = fps.tile([128, 512], F32, tag="fps", name="fps")
            psv = ps[:, 0:cn * 128].rearrange("p (r w) -> p r w", w=128)
            nc.tensor.matmul(psv, pww[:, t * 128:t * 128 + 128],
                             ms[t][:, c0:c0 + cn, :], start=True, stop=True)
            nc.scalar.activation(out=ms[t][:, c0:c0 + cn, :], in_=psv,
                                 func=AF.Copy)
            nc.vector.tensor_scalar_max(
                out=ms[t][:, c0:c0 + cn, :], in0=ms[t][:, c0:c0 + cn, :],
                scalar1=cm[:, t:t + 1])

    def emit_mask_qkv(t, r0, r1):
        nc.vector.tensor_scalar_max(out=qkv[t][:, r0:r1, :],
                                    in0=qkv[t][:, r0:r1, :],
                                    scalar1=cm[:, t:t + 1])

    def emit_kv(r0, r1):
        for r in range(r0, r1):
            tp_ = tps.tile([128, 768], BF16, tag="tp", name="tp")
            for t in range(3):
                nc.tensor.transpose(tp_[:, 128 * t:128 * t + 128],
                                    qkv[t][:, 3 + r, 2:130], idn)
                nc.tensor.transpose(tp_[:, 384 + 128 * t:384 + 128 * t + 128],
                                    ms[t][:, 1 + r, :], idn)
            mT = mtp.tile([128, 768], BF16, tag="mT", name="mT")
            # Pool/GPSIMD cannot read PSUM; DVE is saturated until the last
            # kv rows, so split the psum evacuation Act/DVE by row
            if r < 48:
                nc.scalar.activation(out=mT, in_=tp_, func=AF.Copy)
            else:
                nc.vector.tensor_copy(out=mT, in_=tp_)
            mg = mT.rearrange("p (g c) -> p g c", c=24)
            # matmul stationary operand needs a single free dim: gather k cols
            kc = mtp.tile([128, 256], BF16, tag="kc", name="kc")
            nc.gpsimd.tensor_copy(out=kc.rearrange("p (g e) -> p g e", e=8),
                                  in_=mg[:, :, 8:16])
            for h in range(2):
                kcols = kc[:, 128 * h:128 * h + 128]
                vcols = mg[:, 16 * h:16 * h + 16, 16:24]
                nc.tensor.matmul(kvps[:, 128 * h:128 * h + 128], kcols, vcols,
                                 start=(r == 0), stop=(r == 63),
                                 skip_group_check=True)
                nc.tensor.matmul(kvps[:, 256 + h:257 + h], kcols, ones,
                                 start=(r == 0), stop=(r == 63),
                                 skip_group_check=True)

    # ---- interleaved emission (PE stream must never block on DVE work
    # that gates only LATER PE items) ----
    for k in range(3):                       # conv rows 3..50 (12 chunks)
        for t in (2, 1, 0):
            emit_conv(t, conv_chunks[4 * k:4 * k + 4])
    emit_dwf(0, dw_chunks[0:6])
    emit_dwf(1, dw_chunks[0:6])
    emit_dw5_dve(2, 0, 22)
    emit_dwf(0, dw_chunks[6:11])
    emit_dw5_dve(2, 22, 44)
    emit_dw5_dve(1, 24, 44)
    for k in range(3, 5):                    # conv rows 51..69
        for t in (2, 1, 0):
            emit_conv(t, conv_chunks[4 * k:4 * k + 4])
    emit_pw(2, dw_chunks[0:11])
    emit_dwf(0, dw_chunks[11:17])
    emit_pw(1, dw_chunks[6:11])
    for t in range(3):
        emit_mask_qkv(t, 0, 36)
    emit_kv(0, 33)                           # kv row r needs qkv row 3+r, ms 1+r
    emit_dwf(2, dw_chunks[11:17])
    emit_dwf(1, dw_chunks[11:17])
    for t in range(3):
        emit_mask_qkv(t, 36, 70)
    emit_kv(33, 64)

    # comp extract: [128, 288] = 2 h-blocks of (16 g x 9): col 144h + 9*gl + dd
    comp = env["comp"]
    compv = comp.rearrange("p (h g e) -> p h g e", h=2, e=9)
    kvv = kvps[:, 0:256].rearrange("p (h g e) -> p h g e", h=2, e=8)
    for h in range(2):
        nc.scalar.activation(out=compv[:, h, :, 0:8], in_=kvv[:, h, :, :],
                             func=AF.Copy)
        ksrc = bass.AP(tensor=kvps.tensor, offset=kvps.offset + 256 + h,
                       ap=[[kvps.ap[0][0], 128], [0, 16], [1, 1]])
        nc.scalar.activation(out=compv[:, h, :, 8:9], in_=ksrc, func=AF.Copy)


def _mid(nc, tc, d, env, ctx):
    """AllReduce + scatter + fused att9 + proj + residual -> attf."""
    qkv, ms, attf = env["qkv"], env["ms"], env["attf"]
    pjw, pjb = env["pjw"], env["pjb"]
    comp, compR, wp = env["comp"], env["compR"], env["wp"]
    cc_in, cc_out, rscr = env["cc_in"], env["cc_out"], env["rscr"]

    if DBG:
        nc.sync.dma_start(out=d["dcomp"], in_=comp)
    nc.sync.dma_start(out=cc_in[:], in_=comp)
    nc.gpsimd.collective_compute(
        "AllReduce", Alu.add,
        replica_groups=[[0, 1], [2, 3], [4, 5], [6, 7]],
        ins=[cc_in.opt()], outs=[cc_out.opt()])
    nc.sync.dma_start(out=compR, in_=cc_out[:])
    if DBG:
        nc.sync.dma_start(out=d["dcompR"], in_=compR)

    lsp = ctx.enter_context(tc.tile_pool(name="lsp", bufs=1))
    lhsT_att = {}
    for a in ATT_SRCS:
        for S in ATT_SRCS[a]:
            st = lsp.tile([128, 108], F32, tag=f"lst{a}_{S}", name=f"lst{a}_{S}")
            nc.gpsimd.memset(st, 0.0)
            bt = lsp.tile([128, 108], BF16, tag=f"lat{a}_{S}", name=f"lat{a}_{S}")
            lhsT_att[(a, S)] = [st, bt]
    for g in range(32):
        a, gl9 = g // 12, g % 12
        h, gl = g // 16, g % 16
        S, row0 = q_chan(g, 0) // 128, q_chan(g, 0) % 128
        st = lhsT_att[(a, S)][0]
        L = st.rearrange("p (dd gl) -> p dd gl", gl=12)
        nc.sync.dma_start(
            out=L[row0:row0 + 8, 0:9, gl9:gl9 + 1],
            in_=compR[8 * gl:8 * gl + 8, 144 * h + 9 * gl:144 * h + 9 * gl + 9])
    for key, (st, bt) in lhsT_att.items():
        nc.vector.tensor_copy(out=bt, in_=st)
        lhsT_att[key] = bt

    eb = env["wtile"]("eb", [12, 96], BF16)

    aps = ctx.enter_context(tc.tile_pool(name="aps", bufs=6, space="PSUM"))
    jps = ctx.enter_context(tc.tile_pool(name="jps", bufs=1, space="PSUM"))
    dxp = ctx.enter_context(tc.tile_pool(name="dxp", bufs=1, space="PSUM"))
    dnp = ctx.enter_context(tc.tile_pool(name="dnp", bufs=3))
    arp = ctx.enter_context(tc.tile_pool(name="arp", bufs=4))

    for (c0, cn) in _chunks(0, 66, 4):
        cw = cn * 128

        def att_rhs(S):
            if S < 3:
                return qkv[S][:, 2 + c0:2 + c0 + cn, 2:130]
            return ms[S - 3][:, c0:c0 + cn, :]

        psl = []
        for a in range(3):
            ps = aps.tile([108, 512], F32, tag="aps", name="aps")
            srcs = ATT_SRCS[a]
            for i, S in enumerate(srcs):
                nc.tensor.matmul(ps[:, 0:cw], lhsT_att[(a, S)], att_rhs(S),
                                 start=(i == 0), stop=(i == len(srcs) - 1))
            psl.append(ps)
        den = dnp.tile([12, 3, 512], F32, tag="den", name="den")
        for a in range(3):
            # +eps so padding groups (den == 0) divide to 0, not NaN
            nc.scalar.activation(out=den[0:12, a, 0:cw],
                                 in_=psl[a][96:108, 0:cw], func=AF.Identity,
                                 bias=env["epsv"][0:12, 0:1], scale=1.0)
        rec = dnp.tile([12, 3, 512], BF16, tag="rec", name="rec")
        with nc.allow_low_precision(reason="den recip to bf16"):
            nc.vector.reciprocal(out=rec[:, :, 0:cw], in_=den[:, :, 0:cw])
        jp = jps.tile([128, 512], F32, tag="jps", name="jps")
        dexp = dnp.tile([96, 3, 512], BF16, tag="dexp", name="dexp")
        for a in range(3):
            # broadcast 12 group-divisors to 96 (dd,gl) rows via 0/1 matmul
            dxt = dxp.tile([96, 512], F32, tag="dxt", name="dxt")
            nc.tensor.matmul(dxt[:, 0:cw], eb[0:12, 0:96],
                             rec[:, a, 0:cw], start=True, stop=True)
            nc.scalar.activation(out=dexp[:, a, 0:cw], in_=dxt[:, 0:cw],
                                 func=AF.Copy)
            attc = arp.tile([96, 512], BF16, tag="attc", name="attc")
            nc.vector.tensor_mul(out=attc[:, 0:cw], in0=dexp[:, a, 0:cw],
                                 in1=psl[a][0:96, 0:cw])
            nc.tensor.matmul(jp[:, 0:cw], pjw[0:96, a * 128:a * 128 + 128],
                             attc[:, 0:cw], start=(a == 0), stop=(a == 2))
        attB = arp.tile([128, 512], BF16, tag="attB", name="attB")
        nc.scalar.activation(out=attB[:, 0:cw], in_=jp[:, 0:cw],
                             func=AF.Identity, bias=pjb[:, 0:1], scale=1.0)
        rbt = arp.tile([128, 4, 128], BF16, tag="rbt", name="rbt")
        nc.scalar.dma_start(out=rbt[:, 0:cn, :], in_=d["rb"][:, c0:c0 + cn, :])
        nc.gpsimd.tensor_add(
            out=attf[:, c0 * 128:c0 * 128 + cw], in0=attB[:, 0:cw],
            in1=rbt[:, 0:cn, :].rearrange("p r w -> p (r w)"))


def _tail(nc, tc, d, env, ctx):
    """mb1 + hswish -> h1 (SBUF); dw3 (PE/DVE/Pool) + hswish + mb3 + out."""
    attf = env["attf"]
    if DBG:
        nc.sync.dma_start(out=d["dattf"], in_=attf)
    m1w, m1b, h1b = env["m1w"], env["m1b"], env["h1b"]
    dw3, dw3d, m2b, h2b = env["dw3"], env["dw3d"], env["m2b"], env["h2b"]
    m3w, m3b = env["m3w"], env["m3b"]
    h1 = env["h1"]

    # shared scratch psum: mb1 and dw3-PE outputs round-robin one tag
    sps = ctx.enter_context(tc.tile_pool(name="sps", bufs=3, space="PSUM"))
    m3ps = ctx.enter_context(tc.tile_pool(name="m3ps", bufs=1, space="PSUM"))
    hwp = ctx.enter_context(tc.tile_pool(name="hwp", bufs=3))
    mulp = ctx.enter_context(tc.tile_pool(name="mulp", bufs=7))
    thp = ctx.enter_context(tc.tile_pool(name="thp", bufs=3))
    xap = ctx.enter_context(tc.tile_pool(name="xap", bufs=3))
    accp = ctx.enter_context(tc.tile_pool(name="accp", bufs=3))
    osp = ctx.enter_context(tc.tile_pool(name="osp", bufs=2))

    for t in range(6):
        nc.gpsimd.memset(h1[t][:, :, 0:1], 0.0)
        nc.gpsimd.memset(h1[t][:, :, 129:130], 0.0)
        nc.gpsimd.memset(h1[t][:, 0:1, :], 0.0)

    p8_chunks = _chunks(0, 66, 8)   # 9
    p9_chunks = _chunks(0, 64, 8)   # 8

    def emit_p8(j):
        c0, cn = p8_chunks[j]
        for t in range(6):
            ps = sps.tile([128, 1024], F32, tag="sps", name="sps")
            for (s0, sn) in _chunks(c0, c0 + cn, 4):
                nc.tensor.matmul(
                    ps[:, (s0 - c0) * 128:(s0 - c0 + sn) * 128],
                    m1w[:, t * 128:t * 128 + 128],
                    attf[:, s0 * 128:(s0 + sn) * 128],
                    start=True, stop=True)
            pw_ = ps[:, 0:cn * 128]
            th = hwp.tile([128, 1024], BF16, tag="th", name="th")
            nc.scalar.activation(out=th[:, 0:cn * 128], in_=pw_, func=AF.Relu,
                                 bias=h1b[:, t:t + 1], scale=1.0 / 6.0)
            nc.vector.tensor_scalar_min(out=th[:, 0:cn * 128],
                                        in0=th[:, 0:cn * 128], scalar1=1.0)
            r0 = max(c0, 1)   # keep h1 row 0 zero (dw3 top pad)
            off = (r0 - c0) * 128
            hout = h1[t][:, r0:c0 + cn, 1:129]
            thv = th[:, off:cn * 128].rearrange("p (r w) -> p r w", w=128)
            if t < 4:
                # xh on Act, multiply on Pool (SBUF-only engine)
                xh = hwp.tile([128, 1024], BF16, tag="xh", name="xh")
                nc.scalar.activation(out=xh[:, 0:cn * 128], in_=pw_,
                                     func=AF.Identity, bias=m1b[:, t:t + 1],
                                     scale=1.0)
                nc.gpsimd.tensor_mul(
                    out=hout, in0=thv,
                    in1=xh[:, off:cn * 128].rearrange("p (r w) -> p r w",
                                                      w=128))
            else:
                # fused (psum + m1b) * min(relu(.),1) on DVE
                pv = ps[:, off:cn * 128].rearrange("p (r w) -> p r w", w=128)
                nc.vector.scalar_tensor_tensor(
                    out=hout, in0=pv, scalar=m1b[:, t:t + 1], in1=thv,
                    op0=Alu.add, op1=Alu.mult)

    def emit_p9(j):
        q0, qn = p9_chunks[j]
        mp = m3ps.tile([128, 1024], F32, tag="m3ps", name="m3ps")
        h2fs = [None] * 6
        # pass 1: dw3 + hswish per tile (PE runs ahead across tiles); PE
        # tiles first so their hswish (DVE) unblocks mb3 early, DVE-dw3 last
        for t in (0, 1, 2, 3, 5, 4):
            eng = DW3_ENG[t]
            h2 = mulp.tile([128, 8, 128], BF16, tag="h2", name="h2")
            if eng == "pe":
                ps = sps.tile([128, 1024], F32, tag="sps", name="sps")
                psv = ps[:, 0:qn * 128].rearrange("p (r w) -> p r w", w=128)
                for (s0, sn) in _chunks(0, qn, 4):
                    for tap in range(9):
                        dy, dx = tap // 3, tap % 3
                        k = (t * 9 + tap) * 128
                        nc.tensor.matmul(
                            psv[:, s0:s0 + sn, :], dw3d[:, k:k + 128],
                            h1[t][:, q0 + s0 + dy:q0 + s0 + dy + sn,
                                  dx:dx + 128],
                            start=(tap == 0), stop=(tap == 8))
                th2 = thp.tile([128, 1024], BF16, tag="th2", name="th2")
                nc.scalar.activation(out=th2[:, 0:qn * 128],
                                     in_=ps[:, 0:qn * 128], func=AF.Relu,
                                     bias=h2b[:, t:t + 1], scale=1.0 / 6.0)
                if t < 2:
                    xa = xap.tile([128, 1024], BF16, tag="xa", name="xa")
                    nc.scalar.activation(out=xa[:, 0:qn * 128],
                                         in_=ps[:, 0:qn * 128],
                                         func=AF.Identity,
                                         bias=m2b[:, t:t + 1], scale=1.0)
                    accf = xa[:, 0:qn * 128]
                else:
                    accf = ps[:, 0:qn * 128]   # fused add in STT below
            else:
                acc = accp.tile([128, 8, 128], BF16, tag="acc", name="acc")
                av = acc[:, 0:qn, :]
                for tap in range(9):
                    dy, dx = tap // 3, tap % 3
                    w_ap = dw3[:, t * 9 + tap:t * 9 + tap + 1]
                    win = h1[t][:, q0 + dy:q0 + dy + qn, dx:dx + 128]
                    if eng == "dve":
                        if tap == 0:
                            nc.vector.tensor_scalar(
                                out=av, in0=win, scalar1=w_ap,
                                scalar2=m2b[:, t:t + 1], op0=Alu.mult,
                                op1=Alu.add)
                        else:
                            tmp = accp.tile([128, 8, 128], BF16, tag="tmp",
                                           name="tmp")
                            nc.vector.tensor_scalar_mul(
                                out=tmp[:, 0:qn, :], in0=win, scalar1=w_ap)
                            nc.vector.tensor_add(out=av, in0=av,
                                                 in1=tmp[:, 0:qn, :])
                    else:
                        if tap == 0:
                            nc.gpsimd.tensor_scalar(
                                out=av, in0=win, scalar1=w_ap,
                                scalar2=m2b[:, t:t + 1], op0=Alu.mult,
                                op1=Alu.add)
                        else:
                            nc.gpsimd.scalar_tensor_tensor(
                                out=av, in0=win, scalar=w_ap, in1=av,
                                op0=Alu.mult, op1=Alu.add)
                th2 = thp.tile([128, 1024], BF16, tag="th2", name="th2")
                accf = av.rearrange("p r w -> p (r w)")
                nc.scalar.activation(out=th2[:, 0:qn * 128], in_=accf,
                                     func=AF.Relu, bias=env["half"][:, 0:1],
                                     scale=1.0 / 6.0)
            nc.vector.tensor_scalar_min(out=th2[:, 0:qn * 128],
                                        in0=th2[:, 0:qn * 128], scalar1=1.0)
            h2f = h2.rearrange("p r w -> p (r w)")[:, 0:qn * 128]
            if eng == "pe" and t >= 2:
                nc.vector.scalar_tensor_tensor(
                    out=h2f, in0=accf, scalar=m2b[:, t:t + 1],
                    in1=th2[:, 0:qn * 128], op0=Alu.add, op1=Alu.mult)
            elif t < 2:
                nc.gpsimd.tensor_mul(out=h2f, in0=th2[:, 0:qn * 128],
                                     in1=accf)
            else:
                nc.vector.tensor_mul(out=h2f, in0=th2[:, 0:qn * 128],
                                     in1=accf)
            h2fs[t] = h2f
        # pass 2: mb3 accumulation (DVE-dw3 tile last — ready latest)
        p2 = (0, 1, 2, 3, 5, 4)
        for t in p2:
            for (s0, sn) in _chunks(0, qn, 4):
                nc.tensor.matmul(
                    mp[:, s0 * 128:(s0 + sn) * 128],
                    m3w[:, t * 128:t * 128 + 128],
                    h2fs[t][:, s0 * 128:(s0 + sn) * 128],
                    start=(t == p2[0]), stop=(t == p2[-1]))
        o1 = osp.tile([128, 1024], BF16, tag="o1", name="o1")
        nc.scalar.activation(out=o1[:, 0:qn * 128], in_=mp[:, 0:qn * 128],
                             func=AF.Identity, bias=m3b[:, 0:1], scale=1.0)
        nc.vector.tensor_add(
            out=o1[:, 0:qn * 128], in0=o1[:, 0:qn * 128],
            in1=attf[:, (q0 + 1) * 128:(q0 + 1 + qn) * 128])
        nc.sync.dma_start(
            out=d["out"][:, q0:q0 + qn, :],
            in_=o1[:, 0:qn * 128].rearrange("p (r w) -> p r w", w=128))

    emit_p8(0)
    for j in range(1, 9):
        emit_p8(j)
        emit_p9(j - 1)
    emit_p9(7)
    if DBG:
        for t in range(6):
            nc.sync.dma_start(out=d["dh1"][t], in_=h1[t])


# ====================== host side ======================

def _prep_shared(inp):
    f32 = np.float32
    out = {}
    pw = inp["agg_pw_w"][:, :, 0, 0]
    s1v = inp["bn1_g"] / np.sqrt(inp["bn1_v"] + BN_EPS)
    b1 = inp["bn1_b"] - inp["bn1_m"] * s1v
    Wp = inp["attn_proj_w"][:, :, 0, 0] * s1v[:, None]
    s2 = inp["bn2_g"] / np.sqrt(inp["bn2_v"] + BN_EPS)
    b2 = inp["bn2_b"] - inp["bn2_m"] * s2
    W3 = inp["mb3_w"][:, :, 0, 0] * s2[:, None]
    idn = np.eye(128, dtype=f32)

    for s in (0, 1):
        w = {}
        wc = np.zeros((128, 27 * 128), f32)
        for j, cw in enumerate((inp["wq"], inp["wk"], inp["wv"])):
            for dy in range(3):
                dyy = 2 - dy if s == 1 else dy
                for dx in range(3):
                    k = (j * 9 + dy * 3 + dx) * 128
                    wc[:, k:k + 128] = cw[:, :, dyy, dx].T
        w["wc"] = wc.astype(BF)
        w["cb"] = np.stack([inp["bq"], inp["bk"], inp["bv"]], 1).astype(f32)
        m = np.arange(384)
        w["cm"] = np.where((m % 24) < 16, 0.0, -1e9).astype(f32).reshape(3, 128).T.copy()
        dw5 = np.zeros((128, 75), f32)
        for t in range(3):
            for tap in range(25):
                dy, dx = tap // 5, tap % 5
                dyy = 4 - dy if s == 1 else dy
                dw5[:, t * 25 + tap] = inp["agg_dw_w"][128 * t:128 * t + 128, 0, dyy, dx]
        w["dw5"] = dw5
        # block-diag pw (per tile): pwbd[t][i, o] nonzero iff i//8 == o//8
        pwbd = np.zeros((3, 128, 128), f32)
        for mc in range(384):
            t, o = mc // 128, mc % 128
            g8 = (o // 8) * 8
            pwbd[t, g8:g8 + 8, o] = pw[mc]
        w["pww"] = pwbd.transpose(1, 0, 2).reshape(128, 384).astype(BF)
        # fused dw5+pw: wf[i, (t*25+tap)*128+o] = dw5[i,tap] * pwbd[t][i,o]
        wf = np.zeros((128, 3 * 25 * 128), f32)
        for t in range(3):
            for tap in range(25):
                k = (t * 25 + tap) * 128
                wf[:, k:k + 128] = pwbd[t] * dw5[:, t * 25 + tap][:, None]
        w["wf"] = wf.astype(BF)
        pjw = np.zeros((128, 3 * 128), f32)
        for g in range(32):
            a, gl9 = g // 12, g % 12
            for dd in range(8):
                pjw[12 * dd + gl9, a * 128:a * 128 + 128] = Wp[:, 8 * g + dd]
        w["pjw"] = pjw.astype(BF)
        w["pjb"] = b1.reshape(128, 1).astype(f32)
        m1w = np.zeros((128, 6 * 128), f32)
        for t in range(6):
            m1w[:, t * 128:t * 128 + 128] = inp["mb1_w"][128 * t:128 * t + 128, :, 0, 0].T
        w["m1w"] = m1w.astype(BF)
        w["m1b"] = inp["mb1_b"].reshape(6, 128).T.copy().astype(f32)
        w["h1b"] = (inp["mb1_b"].reshape(6, 128).T / 6.0 + 0.5).astype(f32)
        dw3 = np.zeros((128, 54), f32)
        for t in range(6):
            for tap in range(9):
                dy, dx = tap // 3, tap % 3
                dyy = 2 - dy if s == 1 else dy
                dw3[:, t * 9 + tap] = inp["mb2_w"][128 * t:128 * t + 128, 0, dyy, dx]
        w["dw3"] = dw3
        dw3d = np.zeros((128, 54 * 128), f32)
        for c in range(54):
            dw3d[:, c * 128:c * 128 + 128] = np.diag(dw3[:, c])
        w["dw3d"] = dw3d.astype(BF)
        w["m2b"] = inp["mb2_b"].reshape(6, 128).T.copy().astype(f32)
        w["h2b"] = (inp["mb2_b"].reshape(6, 128).T / 6.0 + 0.5).astype(f32)
        m3w = np.zeros((128, 6 * 128), f32)
        for t in range(6):
            m3w[:, t * 128:t * 128 + 128] = W3[:, 128 * t:128 * t + 128].T
        w["m3w"] = m3w.astype(BF)
        w["m3b"] = b2.reshape(128, 1).astype(f32)
        w["idn"] = idn.astype(BF)
        eb = np.zeros((12, 96), f32)
        for p in range(96):
            eb[p % 12, p] = 1.0
        w["eb"] = eb.astype(BF)
        out[s] = w
    return out


def _prep_core(inp, b, s):
    f32 = np.float32
    ref = inp["ref_features"][b]
    oth = inp["other_features"][b]
    if s == 1:
        ref = ref[:, ::-1, :]
        oth = oth[:, ::-1, :]
    xr = np.zeros((128, 72, 130), f32)
    xo = np.zeros((128, 72, 130), f32)
    xr[:, 4:72, 1:129] = ref[:, 0:68, :]
    xo[:, 4:72, 1:129] = oth[:, 0:68, :]
    rr = np.zeros((128, 66, 128), f32)
    rr[:, 1:66, :] = ref[:, 0:65, :]
    return {"xr": xr.astype(BF), "xo": xo.astype(BF), "rb": rr.astype(BF)}


def kernel(**inputs):
    inp = {k: np.asarray(v) for k, v in inputs.items()}
    if "nc" not in _CACHE:
        _CACHE["nc"] = build_program()
    nc = _CACHE["nc"]
    ws = _prep_shared(inp)
    in_maps = []
    for c in range(NCORES):
        b, s = c // 2, c % 2
        m = dict(ws[s])
        m.update(_prep_core(inp, b, s))
        in_maps.append(m)
    res = bass_utils.run_bass_kernel_spmd(nc, in_maps,
                                          core_ids=list(range(NCORES)))
    out = np.zeros((4, 128, 128, 128), np.float32)
    for c in range(NCORES):
        b, s = c // 2, c % 2
        o = res.results[c]["out"].astype(np.float32)
        if s == 1:
            o = o[:, ::-1, :]
        out[b, :, 64 * s:64 * s + 64, :] = o
    return out


# revision 3
# speedup vs baseline: 1.0017x; 1.0017x over previous
"""EfficientViT attention block on 8 TRN2 NeuronCores.

Sharding: 8 cores = 4 images x 2 row-halves (64 rows each + halos); s=1 cores
get vertically flipped images + dy-flipped conv weights (identical SPMD
program). kv partial sums AllReduced pairwise ([128,288] f32).

v2 vs baseline: all intermediates SBUF-resident (no ms/att/attf/h1 DRAM
round-trips), dw5+grouped-pw fused into block-diagonal PE matmuls for tiles
0-1 (DVE for tile 2), per-row DMA transposes replaced by PE transposes + Pool
evacuation, attention+projection fused per 4-row chunk, dw3 split across
PE/DVE/Pool, bf16 output (host casts to f32).
"""
import contextlib

import numpy as np
import ml_dtypes

import concourse.bass as bass
import concourse.bacc as bacc
import concourse.tile as tile
from concourse import mybir
from concourse import bass_utils

F32 = mybir.dt.float32
BF16 = mybir.dt.bfloat16
Alu = mybir.AluOpType
AF = mybir.ActivationFunctionType
BF = ml_dtypes.bfloat16

BN_EPS = 1e-5
NCORES = 8

_CACHE = {}
DBG = False

ATT_SRCS = {0: (0, 1, 2), 1: (2, 3, 4), 2: (4, 5)}  # att9 tile a -> multi tiles

# ---- engine split knobs ----
# dw5: t0 + t1-rows<24 fused on PE; t1-rows>=24 + t2-rows<36 on DVE;
# t2-rows>=36 on Pool (see _head emission)
DW3_ENG = ("pe", "pe", "pe", "pe", "dve", "pe")  # per mid-tile t


def q_chan(g, e):
    return 24 * g + e if g < 16 else 384 + 24 * (g - 16) + e


def _chunks(lo, hi, step):
    out = []
    r = lo
    while r < hi:
        n = min(step, hi - r)
        out.append((r, n))
        r += n
    return out


def build_program():
    nc = bacc.Bacc("TRN2", target_bir_lowering=False, debug=False,
                   enable_asserts=False, num_devices=NCORES)
    d = {}
    def din(name, shape, dt):
        d[name] = nc.dram_tensor(name, shape, dt, kind="ExternalInput").ap()
    din("xr", [128, 72, 130], BF16)
    din("xo", [128, 72, 130], BF16)
    din("wc", [128, 27 * 128], BF16)
    din("cb", [128, 3], F32)
    din("cm", [128, 3], F32)
    din("wf", [128, 3 * 25 * 128], BF16)
    din("dw5", [128, 75], F32)
    din("pww", [128, 3 * 128], BF16)
    din("pjw", [128, 3 * 128], BF16)
    din("pjb", [128, 1], F32)
    din("m1w", [128, 6 * 128], BF16)
    din("m1b", [128, 6], F32)
    din("h1b", [128, 6], F32)
    din("dw3", [128, 54], F32)
    din("dw3d", [128, 54 * 128], BF16)
    din("m2b", [128, 6], F32)
    din("h2b", [128, 6], F32)
    din("m3w", [128, 6 * 128], BF16)
    din("m3b", [128, 1], F32)
    din("rb", [128, 66, 128], BF16)
    din("idn", [128, 128], BF16)
    din("eb", [12, 96], BF16)
    d["out"] = nc.dram_tensor("out", [128, 64, 128], BF16,
                              kind="ExternalOutput").ap()
    if DBG:
        for nm, sh, dt in (("dqkv", [3, 128, 70, 132], BF16),
                           ("dms", [3, 128, 66, 128], BF16),
                           ("dcomp", [128, 288], F32),
                           ("dcompR", [128, 288], F32),
                           ("dattf", [128, 66 * 128], BF16),
                           ("dh1", [6, 128, 66, 130], BF16)):
            d[nm] = nc.dram_tensor(nm, sh, dt, kind="ExternalOutput").ap()
    with tile.TileContext(nc) as tc:
        _emit(nc, tc, d)
    nc.compile()
    return nc


def _emit(nc, tc, d):
    env = {}
    with contextlib.ExitStack() as ctx:
        wp = ctx.enter_context(tc.tile_pool(name="wp", bufs=1))
        dram = ctx.enter_context(tc.tile_pool(name="dram", bufs=1, space="DRAM"))
        env["wp"] = wp

        def wtile(name, shape, dt, pool=wp):
            t = pool.tile(shape, dt, tag=name, name=name)
            nc.sync.dma_start(out=t, in_=d[name])
            return t
        env["wtile"] = wtile

        for nm, sh, dt in (("pjw", [128, 3 * 128], BF16), ("pjb", [128, 1], F32),
                           ("m1w", [128, 6 * 128], BF16), ("m1b", [128, 6], F32),
                           ("h1b", [128, 6], F32), ("dw3", [128, 54], F32),
                           ("m2b", [128, 6], F32), ("h2b", [128, 6], F32),
                           ("m3w", [128, 6 * 128], BF16), ("m3b", [128, 1], F32)):
            env[nm] = wtile(nm, sh, dt)
        env["attf"] = wp.tile([128, 66 * 128], BF16, tag="attf", name="attf")
        env["half"] = wp.tile([128, 1], F32, tag="half", name="half")
        nc.vector.memset(env["half"], 0.5)
        env["epsv"] = wp.tile([128, 1], F32, tag="epsv", name="epsv")
        nc.vector.memset(env["epsv"], 1e-6)
        env["comp"] = wp.tile([128, 288], F32, tag="comp", name="comp")
        env["compR"] = wp.tile([128, 288], F32, tag="compR", name="compR")
        env["cc_in"] = dram.tile([128, 288], F32, tag="cc_in", name="cc_in")
        env["cc_out"] = dram.tile([128, 288], F32, tag="cc_out", name="cc_out")
        env["rscr"] = dram.tile([12, 1536], BF16, tag="rscr", name="rscr")

        with tc.tile_pool(name="qkvms", bufs=1) as qp:
            env["qkv"] = [qp.tile([128, 70, 132], BF16, tag=f"qkv{t}",
                                  name=f"qkv{t}") for t in range(3)]
            env["ms"] = [qp.tile([128, 66, 128], BF16, tag=f"ms{t}",
                                 name=f"ms{t}") for t in range(3)]
            with contextlib.ExitStack() as hctx:
                _head(nc, tc, d, env, hctx)
            with contextlib.ExitStack() as mctx:
                _mid(nc, tc, d, env, mctx)
        with tc.tile_pool(name="tailp", bufs=1) as tp:
            env["h1"] = [tp.tile([128, 66, 130], BF16, tag=f"h1_{t}",
                                 name=f"h1_{t}") for t in range(6)]
            env["dw3d"] = wtile("dw3d", [128, 54 * 128], BF16, pool=tp)
            with contextlib.ExitStack() as tctx:
                _tail(nc, tc, d, env, tctx)


def _head(nc, tc, d, env, ctx):
    """conv qkv + dw5/pw (ms) + per-row transposes + kv accumulation."""
    qkv, ms = env["qkv"], env["ms"]
    wtile = env["wtile"]

    xp = ctx.enter_context(tc.tile_pool(name="xp", bufs=1))
    wc = wtile("wc", [128, 27 * 128], BF16, pool=xp)
    xr = xp.tile([128, 72, 130], BF16, tag="xr", name="xr")
    xo = xp.tile([128, 72, 130], BF16, tag="xo", name="xo")
    for (r0, rn) in ((0, 28), (28, 22), (50, 22)):
        nc.sync.dma_start(out=xr[:, r0:r0 + rn, :], in_=d["xr"][:, r0:r0 + rn, :])
        nc.sync.dma_start(out=xo[:, r0:r0 + rn, :], in_=d["xo"][:, r0:r0 + rn, :])
    wf = wtile("wf", [128, 3 * 25 * 128], BF16, pool=xp)
    cb = wtile("cb", [128, 3], F32, pool=xp)
    cm = wtile("cm", [128, 3], F32, pool=xp)
    dw5 = wtile("dw5", [128, 75], F32, pool=xp)
    pww = wtile("pww", [128, 3 * 128], BF16, pool=xp)
    idn = wtile("idn", [128, 128], BF16, pool=xp)
    ones = xp.tile([128, 1], BF16, tag="ones", name="ones")
    nc.vector.memset(ones, 1.0)
    env["cm"] = cm

    # pad memsets (cols 0:2 / 130:132 and top rows 0:3 of each qkv tile)
    for t in range(3):
        nc.vector.memset(qkv[t][:, :, 0:2], 0.0)
        nc.vector.memset(qkv[t][:, :, 130:132], 0.0)
        nc.vector.memset(qkv[t][:, 0:3, :], 0.0)

    cps = ctx.enter_context(tc.tile_pool(name="cps", bufs=2, space="PSUM"))
    fps = ctx.enter_context(tc.tile_pool(name="fps", bufs=2, space="PSUM"))
    tps = ctx.enter_context(tc.tile_pool(name="tps", bufs=3, space="PSUM"))
    kps = ctx.enter_context(tc.tile_pool(name="kps", bufs=1, space="PSUM"))
    mtp = ctx.enter_context(tc.tile_pool(name="mtp", bufs=3))
    dtp = ctx.enter_context(tc.tile_pool(name="dtp", bufs=1))
    app = ctx.enter_context(tc.tile_pool(name="app", bufs=1))

    kvps = kps.tile([128, 288], F32, tag="kvps", name="kvps")

    conv_chunks = _chunks(3, 70, 4)            # qkv tile rows (17)
    dw_chunks = _chunks(0, 66, 4)              # ms rows (17)

    def emit_conv(t, chunks):
        src = xr if t == 0 else xo
        for (r0, rn) in chunks:
            ps = cps.tile([128, 512], F32, tag="cps", name="cps")
            psv = ps[:, 0:rn * 128].rearrange("p (r w) -> p r w", w=128)
            first = True
            for dy in range(3):
                for dx in range(3):
                    k = (t * 9 + dy * 3 + dx) * 128
                    nc.tensor.matmul(
                        psv, wc[:, k:k + 128],
                        src[:, r0 + dy:r0 + dy + rn, dx:dx + 128],
                        start=first, stop=(dy == 2 and dx == 2))
                    first = False
            nc.scalar.activation(
                out=qkv[t][:, r0:r0 + rn, 2:130], in_=psv,
                func=AF.Identity, bias=cb[:, t:t + 1], scale=1.0)

    def emit_dwf(t, chunks):
        # fused dw5x5 + grouped-pw on PE: 25 block-diag matmuls per chunk
        for (c0, cn) in chunks:
            ps = fps.tile([128, 512], F32, tag="fps", name="fps")
            psv = ps[:, 0:cn * 128].rearrange("p (r w) -> p r w", w=128)
            for tap in range(25):
                dy, dx = tap // 5, tap % 5
                k = (t * 25 + tap) * 128
                nc.tensor.matmul(
                    psv, wf[:, k:k + 128],
                    qkv[t][:, c0 + dy:c0 + dy + cn, dx:dx + 128],
                    start=(tap == 0), stop=(tap == 24))
            nc.scalar.activation(out=ms[t][:, c0:c0 + cn, :], in_=psv,
                                 func=AF.Copy)
            nc.vector.tensor_scalar_max(
                out=ms[t][:, c0:c0 + cn, :], in0=ms[t][:, c0:c0 + cn, :],
                scalar1=cm[:, t:t + 1])

    def emit_dw5_dve(t, o0, o1):
        # dw5 via TS-mul + TT-add on DVE, acc in-place in ms[t]
        for (r0, n) in _chunks(o0, o1, 22):
            acc = ms[t][:, r0:r0 + n, :]
            for tap in range(25):
                dy, dx = tap // 5, tap % 5
                w_ap = dw5[:, t * 25 + tap:t * 25 + tap + 1]
                win = qkv[t][:, r0 + dy:r0 + dy + n, dx:dx + 128]
                if tap == 0:
                    nc.vector.tensor_scalar_mul(out=acc, in0=win, scalar1=w_ap)
                else:
                    tmp = dtp.tile([128, 22, 128], BF16, tag="dwtmp",
                                   name="dwtmp")
                    nc.vector.tensor_scalar_mul(out=tmp[:, 0:n, :], in0=win,
                                                scalar1=w_ap)
                    nc.vector.tensor_add(out=acc, in0=acc, in1=tmp[:, 0:n, :])

    def emit_dw5_ap(t, o0, o1, taps):
        # dw5 with multiplies on DVE (cheap 4x mode) and adds on Pool
        for (r0, n) in _chunks(o0, o1, 22):
            acc = ms[t][:, r0:r0 + n, :]
            for tap in taps:
                dy, dx = tap // 5, tap % 5
                w_ap = dw5[:, t * 25 + tap:t * 25 + tap + 1]
                win = qkv[t][:, r0 + dy:r0 + dy + n, dx:dx + 128]
                if tap == 0:
                    nc.vector.tensor_scalar_mul(out=acc, in0=win, scalar1=w_ap)
                else:
                    tmp = app.tile([128, 22, 128], BF16, tag="aptmp",
                                   name="aptmp")
                    nc.vector.tensor_scalar_mul(out=tmp[:, 0:n, :], in0=win,
                                                scalar1=w_ap)
                    nc.gpsimd.tensor_add(out=acc, in0=acc, in1=tmp[:, 0:n, :])

    def emit_pw(t, chunks):
        # grouped pw over raw dw5 acc already in ms[t]; evac back in place
        for (c0, cn) in chunks:
            ps = fps.tile([128, 512], F32, tag="fps", name="fps")
            psv = ps[:, 0:cn * 128].rearrange("p (r w) -> p r w", w=128)
            nc.tensor.matmul(psv, pww[:, t * 128:t * 128 + 128],
                             ms[t][:, c0:c0 + cn, :], start=True, stop=True)
            nc.scalar.activation(out=ms[t][:, c0:c0 + cn, :], in_=psv,
                                 func=AF.Copy)
            nc.vector.tensor_scalar_max(
                out=ms[t][:, c0:c0 + cn, :], in0=ms[t][:, c0:c0 + cn, :],
                scalar1=cm[:, t:t + 1])

    def emit_mask_qkv(t, r0, r1):
        nc.vector.tensor_scalar_max(out=qkv[t][:, r0:r1, :],
                                    in0=qkv[t][:, r0:r1, :],
                                    scalar1=cm[:, t:t + 1])

    def emit_kv(r0, r1):
        for r in range(r0, r1):
            tp_ = tps.tile([128, 768], BF16, tag="tp", name="tp")
            for t in range(3):
                nc.tensor.transpose(tp_[:, 128 * t:128 * t + 128],
                                    qkv[t][:, 3 + r, 2:130], idn)
                nc.tensor.transpose(tp_[:, 384 + 128 * t:384 + 128 * t + 128],
                                    ms[t][:, 1 + r, :], idn)
            mT = mtp.tile([128, 768], BF16, tag="mT", name="mT")
            # Pool/GPSIMD cannot read PSUM; DVE is saturated until the last
            # kv rows, so split the psum evacuation Act/DVE by row
            if r < 48:
                nc.scalar.activation(out=mT, in_=tp_, func=AF.Copy)
            else:
                nc.vector.tensor_copy(out=mT, in_=tp_)
            mg = mT.rearrange("p (g c) -> p g c", c=24)
            # matmul stationary operand needs a single free dim: gather k cols
            kc = mtp.tile([128, 256], BF16, tag="kc", name="kc")
            nc.gpsimd.tensor_copy(out=kc.rearrange("p (g e) -> p g e", e=8),
                                  in_=mg[:, :, 8:16])
            for h in range(2):
                kcols = kc[:, 128 * h:128 * h + 128]
                vcols = mg[:, 16 * h:16 * h + 16, 16:24]
                nc.tensor.matmul(kvps[:, 128 * h:128 * h + 128], kcols, vcols,
                                 start=(r == 0), stop=(r == 63),
                                 skip_group_check=True)
                nc.tensor.matmul(kvps[:, 256 + h:257 + h], kcols, ones,
                                 start=(r == 0), stop=(r == 63),
                                 skip_group_check=True)

    # ---- interleaved emission (PE stream must never block on DVE work
    # that gates only LATER PE items) ----
    for k in range(3):                       # conv rows 3..50 (12 chunks)
        for t in (2, 1, 0):
            emit_conv(t, conv_chunks[4 * k:4 * k + 4])
    emit_dwf(0, dw_chunks[0:6])
    emit_dwf(1, dw_chunks[0:6])
    emit_dw5_dve(2, 0, 22)
    emit_dwf(0, dw_chunks[6:11])
    emit_dw5_dve(2, 22, 44)
    emit_dw5_dve(1, 24, 44)
    for k in range(3, 5):                    # conv rows 51..69
        for t in (2, 1, 0):
            emit_conv(t, conv_chunks[4 * k:4 * k + 4])
    emit_pw(2, dw_chunks[0:11])
    emit_dwf(0, dw_chunks[11:17])
    emit_pw(1, dw_chunks[6:11])
    for t in range(3):
        emit_mask_qkv(t, 0, 36)
    emit_kv(0, 33)                           # kv row r needs qkv row 3+r, ms 1+r
    emit_dwf(2, dw_chunks[11:17])
    emit_dwf(1, dw_chunks[11:17])
    for t in range(3):
        emit_mask_qkv(t, 36, 70)
    emit_kv(33, 64)

    # comp extract: [128, 288] = 2 h-blocks of (16 g x 9): col 144h + 9*gl + dd
    comp = env["comp"]
    compv = comp.rearrange("p (h g e) -> p h g e", h=2, e=9)
    kvv = kvps[:, 0:256].rearrange("p (h g e) -> p h g e", h=2, e=8)
    for h in range(2):
        nc.scalar.activation(out=compv[:, h, :, 0:8], in_=kvv[:, h, :, :],
                             func=AF.Copy)
        ksrc = bass.AP(tensor=kvps.tensor, offset=kvps.offset + 256 + h,
                       ap=[[kvps.ap[0][0], 128], [0, 16], [1, 1]])
        nc.scalar.activation(out=compv[:, h, :, 8:9], in_=ksrc, func=AF.Copy)


def _mid(nc, tc, d, env, ctx):
    """AllReduce + scatter + fused att9 + proj + residual -> attf."""
    qkv, ms, attf = env["qkv"], env["ms"], env["attf"]
    pjw, pjb = env["pjw"], env["pjb"]
    comp, compR, wp = env["comp"], env["compR"], env["wp"]
    cc_in, cc_out, rscr = env["cc_in"], env["cc_out"], env["rscr"]

    if DBG:
        nc.sync.dma_start(out=d["dcomp"], in_=comp)
    nc.sync.dma_start(out=cc_in[:], in_=comp)
    nc.gpsimd.collective_compute(
        "AllReduce", Alu.add,
        replica_groups=[[0, 1], [2, 3], [4, 5], [6, 7]],
        ins=[cc_in.opt()], outs=[cc_out.opt()])
    nc.sync.dma_start(out=compR, in_=cc_out[:])
    if DBG:
        nc.sync.dma_start(out=d["dcompR"], in_=compR)

    lsp = ctx.enter_context(tc.tile_pool(name="lsp", bufs=1))
    lhsT_att = {}
    for a in ATT_SRCS:
        for S in ATT_SRCS[a]:
            st = lsp.tile([128, 108], F32, tag=f"lst{a}_{S}", name=f"lst{a}_{S}")
            nc.gpsimd.memset(st, 0.0)
            bt = lsp.tile([128, 108], BF16, tag=f"lat{a}_{S}", name=f"lat{a}_{S}")
            lhsT_att[(a, S)] = [st, bt]
    for g in range(32):
        a, gl9 = g // 12, g % 12
        h, gl = g // 16, g % 16
        S, row0 = q_chan(g, 0) // 128, q_chan(g, 0) % 128
        st = lhsT_att[(a, S)][0]
        L = st.rearrange("p (dd gl) -> p dd gl", gl=12)
        nc.sync.dma_start(
            out=L[row0:row0 + 8, 0:9, gl9:gl9 + 1],
            in_=compR[8 * gl:8 * gl + 8, 144 * h + 9 * gl:144 * h + 9 * gl + 9])
    for key, (st, bt) in lhsT_att.items():
        nc.vector.tensor_copy(out=bt, in_=st)
        lhsT_att[key] = bt

    eb = env["wtile"]("eb", [12, 96], BF16)

    aps = ctx.enter_context(tc.tile_pool(name="aps", bufs=6, space="PSUM"))
    jps = ctx.enter_context(tc.tile_pool(name="jps", bufs=1, space="PSUM"))
    dxp = ctx.enter_context(tc.tile_pool(name="dxp", bufs=1, space="PSUM"))
    dnp = ctx.enter_context(tc.tile_pool(name="dnp", bufs=3))
    arp = ctx.enter_context(tc.tile_pool(name="arp", bufs=4))

    for (c0, cn) in _chunks(0, 66, 4):
        cw = cn * 128

        def att_rhs(S):
            if S < 3:
                return qkv[S][:, 2 + c0:2 + c0 + cn, 2:130]
            return ms[S - 3][:, c0:c0 + cn, :]

        psl = []
        for a in range(3):
            ps = aps.tile([108, 512], F32, tag="aps", name="aps")
            srcs = ATT_SRCS[a]
            for i, S in enumerate(srcs):
                nc.tensor.matmul(ps[:, 0:cw], lhsT_att[(a, S)], att_rhs(S),
                                 start=(i == 0), stop=(i == len(srcs) - 1))
            psl.append(ps)
        den = dnp.tile([12, 3, 512], F32, tag="den", name="den")
        for a in range(3):
            # +eps so padding groups (den == 0) divide to 0, not NaN
            nc.scalar.activation(out=den[0:12, a, 0:cw],
                                 in_=psl[a][96:108, 0:cw], func=AF.Identity,
                                 bias=env["epsv"][0:12, 0:1], scale=1.0)
        rec = dnp.tile([12, 3, 512], BF16, tag="rec", name="rec")
        with nc.allow_low_precision(reason="den recip to bf16"):
            nc.vector.reciprocal(out=rec[:, :, 0:cw], in_=den[:, :, 0:cw])
        jp = jps.tile([128, 512], F32, tag="jps", name="jps")
        dexp = dnp.tile([96, 3, 512], BF16, tag="dexp", name="dexp")
        for a in range(3):
            # broadcast 12 group-divisors to 96 (dd,gl) rows via 0/1 matmul
            dxt = dxp.tile([96, 512], F32, tag="dxt", name="dxt")
            nc.tensor.matmul(dxt[:, 0:cw], eb[0:12, 0:96],
                             rec[:, a, 0:cw], start=True, stop=True)
            nc.scalar.activation(out=dexp[:, a, 0:cw], in_=dxt[:, 0:cw],
                                 func=AF.Copy)
            attc = arp.tile([96, 512], BF16, tag="attc", name="attc")
            nc.vector.tensor_mul(out=attc[:, 0:cw], in0=dexp[:, a, 0:cw],
                                 in1=psl[a][0:96, 0:cw])
            nc.tensor.matmul(jp[:, 0:cw], pjw[0:96, a * 128:a * 128 + 128],
                             attc[:, 0:cw], start=(a == 0), stop=(a == 2))
        attB = arp.tile([128, 512], BF16, tag="attB", name="attB")
        nc.scalar.activation(out=attB[:, 0:cw], in_=jp[:, 0:cw],
                             func=AF.Identity, bias=pjb[:, 0:1], scale=1.0)
        rbt = arp.tile([128, 4, 128], BF16, tag="rbt", name="rbt")
        nc.scalar.dma_start(out=rbt[:, 0:cn, :], in_=d["rb"][:, c0:c0 + cn, :])
        nc.gpsimd.tensor_add(
            out=attf[:, c0 * 128:c0 * 128 + cw], in0=attB[:, 0:cw],
            in1=rbt[:, 0:cn, :].rearrange("p r w -> p (r w)"))


def _tail(nc, tc, d, env, ctx):
    """mb1 + hswish -> h1 (SBUF); dw3 (PE/DVE/Pool) + hswish + mb3 + out."""
    attf = env["attf"]
    if DBG:
        nc.sync.dma_start(out=d["dattf"], in_=attf)
    m1w, m1b, h1b = env["m1w"], env["m1b"], env["h1b"]
    dw3, dw3d, m2b, h2b = env["dw3"], env["dw3d"], env["m2b"], env["h2b"]
    m3w, m3b = env["m3w"], env["m3b"]
    h1 = env["h1"]

    # shared scratch psum: mb1 and dw3-PE outputs round-robin one tag
    sps = ctx.enter_context(tc.tile_pool(name="sps", bufs=3, space="PSUM"))
    m3ps = ctx.enter_context(tc.tile_pool(name="m3ps", bufs=1, space="PSUM"))
    hwp = ctx.enter_context(tc.tile_pool(name="hwp", bufs=4))
    mulp = ctx.enter_context(tc.tile_pool(name="mulp", bufs=7))
    thp = ctx.enter_context(tc.tile_pool(name="thp", bufs=4))
    xap = ctx.enter_context(tc.tile_pool(name="xap", bufs=3))
    accp = ctx.enter_context(tc.tile_pool(name="accp", bufs=3))
    osp = ctx.enter_context(tc.tile_pool(name="osp", bufs=2))

    for t in range(6):
        nc.gpsimd.memset(h1[t][:, :, 0:1], 0.0)
        nc.gpsimd.memset(h1[t][:, :, 129:130], 0.0)
        nc.gpsimd.memset(h1[t][:, 0:1, :], 0.0)

    p8_chunks = _chunks(0, 66, 8)   # 9
    p9_chunks = _chunks(0, 64, 8)   # 8

    def emit_p8(j):
        c0, cn = p8_chunks[j]
        for t in range(6):
            ps = sps.tile([128, 1024], F32, tag="sps", name="sps")
            for (s0, sn) in _chunks(c0, c0 + cn, 4):
                nc.tensor.matmul(
                    ps[:, (s0 - c0) * 128:(s0 - c0 + sn) * 128],
                    m1w[:, t * 128:t * 128 + 128],
                    attf[:, s0 * 128:(s0 + sn) * 128],
                    start=True, stop=True)
            pw_ = ps[:, 0:cn * 128]
            th = hwp.tile([128, 1024], BF16, tag="th", name="th")
            nc.scalar.activation(out=th[:, 0:cn * 128], in_=pw_, func=AF.Relu,
                                 bias=h1b[:, t:t + 1], scale=1.0 / 6.0)
            nc.vector.tensor_scalar_min(out=th[:, 0:cn * 128],
                                        in0=th[:, 0:cn * 128], scalar1=1.0)
            r0 = max(c0, 1)   # keep h1 row 0 zero (dw3 top pad)
            off = (r0 - c0) * 128
            hout = h1[t][:, r0:c0 + cn, 1:129]
            thv = th[:, off:cn * 128].rearrange("p (r w) -> p r w", w=128)
            if t < 4:
                # xh on Act, multiply on Pool (SBUF-only engine)
                xh = hwp.tile([128, 1024], BF16, tag="xh", name="xh")
                nc.scalar.activation(out=xh[:, 0:cn * 128], in_=pw_,
                                     func=AF.Identity, bias=m1b[:, t:t + 1],
                                     scale=1.0)
                nc.gpsimd.tensor_mul(
                    out=hout, in0=thv,
                    in1=xh[:, off:cn * 128].rearrange("p (r w) -> p r w",
                                                      w=128))
            else:
                # fused (psum + m1b) * min(relu(.),1) on DVE
                pv = ps[:, off:cn * 128].rearrange("p (r w) -> p r w", w=128)
                nc.vector.scalar_tensor_tensor(
                    out=hout, in0=pv, scalar=m1b[:, t:t + 1], in1=thv,
                    op0=Alu.add, op1=Alu.mult)

    def emit_p9(j):
        q0, qn = p9_chunks[j]
        mp = m3ps.tile([128, 1024], F32, tag="m3ps", name="m3ps")
        h2fs = [None] * 6
        # pass 1: dw3 + hswish per tile (PE runs ahead across tiles); PE
        # tiles first so their hswish (DVE) unblocks mb3 early, DVE-dw3 last
        for t in (0, 1, 2, 3, 5, 4):
            eng = DW3_ENG[t]
            h2 = mulp.tile([128, 8, 128], BF16, tag="h2", name="h2")
            if eng == "pe":
                ps = sps.tile([128, 1024], F32, tag="sps", name="sps")
                psv = ps[:, 0:qn * 128].rearrange("p (r w) -> p r w", w=128)
                for (s0, sn) in _chunks(0, qn, 4):
                    for tap in range(9):
                        dy, dx = tap // 3, tap % 3
                        k = (t * 9 + tap) * 128
                        nc.tensor.matmul(
                            psv[:, s0:s0 + sn, :], dw3d[:, k:k + 128],
                            h1[t][:, q0 + s0 + dy:q0 + s0 + dy + sn,
                                  dx:dx + 128],
                            start=(tap == 0), stop=(tap == 8))
                th2 = thp.tile([128, 1024], BF16, tag="th2", name="th2")
                nc.scalar.activation(out=th2[:, 0:qn * 128],
                                     in_=ps[:, 0:qn * 128], func=AF.Relu,
                                     bias=h2b[:, t:t + 1], scale=1.0 / 6.0)
                if t < 2:
                    xa = xap.tile([128, 1024], BF16, tag="xa", name="xa")
                    nc.scalar.activation(out=xa[:, 0:qn * 128],
                                         in_=ps[:, 0:qn * 128],
                                         func=AF.Identity,
                                         bias=m2b[:, t:t + 1], scale=1.0)
                    accf = xa[:, 0:qn * 128]
                else:
                    accf = ps[:, 0:qn * 128]   # fused add in STT below
            else:
                acc = accp.tile([128, 8, 128], BF16, tag="acc", name="acc")
                av = acc[:, 0:qn, :]
                for tap in range(9):
                    dy, dx = tap // 3, tap % 3
                    w_ap = dw3[:, t * 9 + tap:t * 9 + tap + 1]
                    win = h1[t][:, q0 + dy:q0 + dy + qn, dx:dx + 128]
                    if eng == "dve":
                        if tap == 0:
                            nc.vector.tensor_scalar(
                                out=av, in0=win, scalar1=w_ap,
                                scalar2=m2b[:, t:t + 1], op0=Alu.mult,
                                op1=Alu.add)
                        else:
                            tmp = accp.tile([128, 8, 128], BF16, tag="tmp",
                                           name="tmp")
                            nc.vector.tensor_scalar_mul(
                                out=tmp[:, 0:qn, :], in0=win, scalar1=w_ap)
                            nc.vector.tensor_add(out=av, in0=av,
                                                 in1=tmp[:, 0:qn, :])
                    else:
                        if tap == 0:
                            nc.gpsimd.tensor_scalar(
                                out=av, in0=win, scalar1=w_ap,
                                scalar2=m2b[:, t:t + 1], op0=Alu.mult,
                                op1=Alu.add)
                        else:
                            nc.gpsimd.scalar_tensor_tensor(
                                out=av, in0=win, scalar=w_ap, in1=av,
                                op0=Alu.mult, op1=Alu.add)
                th2 = thp.tile([128, 1024], BF16, tag="th2", name="th2")
                accf = av.rearrange("p r w -> p (r w)")
                nc.scalar.activation(out=th2[:, 0:qn * 128], in_=accf,
                                     func=AF.Relu, bias=env["half"][:, 0:1],
                                     scale=1.0 / 6.0)
            nc.vector.tensor_scalar_min(out=th2[:, 0:qn * 128],
                                        in0=th2[:, 0:qn * 128], scalar1=1.0)
            h2f = h2.rearrange("p r w -> p (r w)")[:, 0:qn * 128]
            if eng == "pe" and t >= 2:
                nc.vector.scalar_tensor_tensor(
                    out=h2f, in0=accf, scalar=m2b[:, t:t + 1],
                    in1=th2[:, 0:qn * 128], op0=Alu.add, op1=Alu.mult)
            elif t < 2:
                nc.gpsimd.tensor_mul(out=h2f, in0=th2[:, 0:qn * 128],
                                     in1=accf)
            else:
                nc.vector.tensor_mul(out=h2f, in0=th2[:, 0:qn * 128],
                                     in1=accf)
            h2fs[t] = h2f
        # pass 2: mb3 accumulation (DVE-dw3 tile last — ready latest)
        p2 = (0, 1, 2, 3, 5, 4)
        for t in p2:
            for (s0, sn) in _chunks(0, qn, 4):
                nc.tensor.matmul(
                    mp[:, s0 * 128:(s0 + sn) * 128],
                    m3w[:, t * 128:t * 128 + 128],
                    h2fs[t][:, s0 * 128:(s0 + sn) * 128],
                    start=(t == p2[0]), stop=(t == p2[-1]))
        o1 = osp.tile([128, 1024], BF16, tag="o1", name="o1")
        nc.scalar.activation(out=o1[:, 0:qn * 128], in_=mp[:, 0:qn * 128],
                             func=AF.Identity, bias=m3b[:, 0:1], scale=1.0)
        nc.vector.tensor_add(
            out=o1[:, 0:qn * 128], in0=o1[:, 0:qn * 128],
            in1=attf[:, (q0 + 1) * 128:(q0 + 1 + qn) * 128])
        nc.sync.dma_start(
            out=d["out"][:, q0:q0 + qn, :],
            in_=o1[:, 0:qn * 128].rearrange("p (r w) -> p r w", w=128))

    emit_p8(0)
    for j in range(1, 9):
        emit_p8(j)
        emit_p9(j - 1)
    emit_p9(7)
    if DBG:
        for t in range(6):
            nc.sync.dma_start(out=d["dh1"][t], in_=h1[t])


# ====================== host side ======================

def _prep_shared(inp):
    f32 = np.float32
    out = {}
    pw = inp["agg_pw_w"][:, :, 0, 0]
    s1v = inp["bn1_g"] / np.sqrt(inp["bn1_v"] + BN_EPS)
    b1 = inp["bn1_b"] - inp["bn1_m"] * s1v
    Wp = inp["attn_proj_w"][:, :, 0, 0] * s1v[:, None]
    s2 = inp["bn2_g"] / np.sqrt(inp["bn2_v"] + BN_EPS)
    b2 = inp["bn2_b"] - inp["bn2_m"] * s2
    W3 = inp["mb3_w"][:, :, 0, 0] * s2[:, None]
    idn = np.eye(128, dtype=f32)

    for s in (0, 1):
        w = {}
        wc = np.zeros((128, 27 * 128), f32)
        for j, cw in enumerate((inp["wq"], inp["wk"], inp["wv"])):
            for dy in range(3):
                dyy = 2 - dy if s == 1 else dy
                for dx in range(3):
                    k = (j * 9 + dy * 3 + dx) * 128
                    wc[:, k:k + 128] = cw[:, :, dyy, dx].T
        w["wc"] = wc.astype(BF)
        w["cb"] = np.stack([inp["bq"], inp["bk"], inp["bv"]], 1).astype(f32)
        m = np.arange(384)
        w["cm"] = np.where((m % 24) < 16, 0.0, -1e9).astype(f32).reshape(3, 128).T.copy()
        dw5 = np.zeros((128, 75), f32)
        for t in range(3):
            for tap in range(25):
                dy, dx = tap // 5, tap % 5
                dyy = 4 - dy if s == 1 else dy
                dw5[:, t * 25 + tap] = inp["agg_dw_w"][128 * t:128 * t + 128, 0, dyy, dx]
        w["dw5"] = dw5
        # block-diag pw (per tile): pwbd[t][i, o] nonzero iff i//8 == o//8
        pwbd = np.zeros((3, 128, 128), f32)
        for mc in range(384):
            t, o = mc // 128, mc % 128
            g8 = (o // 8) * 8
            pwbd[t, g8:g8 + 8, o] = pw[mc]
        w["pww"] = pwbd.transpose(1, 0, 2).reshape(128, 384).astype(BF)
        # fused dw5+pw: wf[i, (t*25+tap)*128+o] = dw5[i,tap] * pwbd[t][i,o]
        wf = np.zeros((128, 3 * 25 * 128), f32)
        for t in range(3):
            for tap in range(25):
                k = (t * 25 + tap) * 128
                wf[:, k:k + 128] = pwbd[t] * dw5[:, t * 25 + tap][:, None]
        w["wf"] = wf.astype(BF)
        pjw = np.zeros((128, 3 * 128), f32)
        for g in range(32):
            a, gl9 = g // 12, g % 12
            for dd in range(8):
                pjw[12 * dd + gl9, a * 128:a * 128 + 128] = Wp[:, 8 * g + dd]
        w["pjw"] = pjw.astype(BF)
        w["pjb"] = b1.reshape(128, 1).astype(f32)
        m1w = np.zeros((128, 6 * 128), f32)
        for t in range(6):
            m1w[:, t * 128:t * 128 + 128] = inp["mb1_w"][128 * t:128 * t + 128, :, 0, 0].T
        w["m1w"] = m1w.astype(BF)
        w["m1b"] = inp["mb1_b"].reshape(6, 128).T.copy().astype(f32)
        w["h1b"] = (inp["mb1_b"].reshape(6, 128).T / 6.0 + 0.5).astype(f32)
        dw3 = np.zeros((128, 54), f32)
        for t in range(6):
            for tap in range(9):
                dy, dx = tap // 3, tap % 3
                dyy = 2 - dy if s == 1 else dy
                dw3[:, t * 9 + tap] = inp["mb2_w"][128 * t:128 * t + 128, 0, dyy, dx]
        w["dw3"] = dw3
        dw3d = np.zeros((128, 54 * 128), f32)
        for c in range(54):
            dw3d[:, c * 128:c * 128 + 128] = np.diag(dw3[:, c])
        w["dw3d"] = dw3d.astype(BF)
        w["m2b"] = inp["mb2_b"].reshape(6, 128).T.copy().astype(f32)
        w["h2b"] = (inp["mb2_b"].reshape(6, 128).T / 6.0 + 0.5).astype(f32)
        m3w = np.zeros((128, 6 * 128), f32)
        for t in range(6):
            m3w[:, t * 128:t * 128 + 128] = W3[:, 128 * t:128 * t + 128].T
        w["m3w"] = m3w.astype(BF)
        w["m3b"] = b2.reshape(128, 1).astype(f32)
        w["idn"] = idn.astype(BF)
        eb = np.zeros((12, 96), f32)
        for p in range(96):
            eb[p % 12, p] = 1.0
        w["eb"] = eb.astype(BF)
        out[s] = w
    return out


def _prep_core(inp, b, s):
    f32 = np.float32
    ref = inp["ref_features"][b]
    oth = inp["other_features"][b]
    if s == 1:
        ref = ref[:, ::-1, :]
        oth = oth[:, ::-1, :]
    xr = np.zeros((128, 72, 130), f32)
    xo = np.zeros((128, 72, 130), f32)
    xr[:, 4:72, 1:129] = ref[:, 0:68, :]
    xo[:, 4:72, 1:129] = oth[:, 0:68, :]
    rr = np.zeros((128, 66, 128), f32)
    rr[:, 1:66, :] = ref[:, 0:65, :]
    return {"xr": xr.astype(BF), "xo": xo.astype(BF), "rb": rr.astype(BF)}


def kernel(**inputs):
    inp = {k: np.asarray(v) for k, v in inputs.items()}
    if "nc" not in _CACHE:
        _CACHE["nc"] = build_program()
    nc = _CACHE["nc"]
    ws = _prep_shared(inp)
    in_maps = []
    for c in range(NCORES):
        b, s = c // 2, c % 2
        m = dict(ws[s])
        m.update(_prep_core(inp, b, s))
        in_maps.append(m)
    res = bass_utils.run_bass_kernel_spmd(nc, in_maps,
                                          core_ids=list(range(NCORES)))
    out = np.zeros((4, 128, 128, 128), np.float32)
    for c in range(NCORES):
        b, s = c // 2, c % 2
        o = res.results[c]["out"].astype(np.float32)
        if s == 1:
            o = o[:, ::-1, :]
        out[b, :, 64 * s:64 * s + 64, :] = o
    return out


# revision 4
# speedup vs baseline: 1.0276x; 1.0259x over previous
"""EfficientViT attention block on 8 TRN2 NeuronCores.

Sharding: 8 cores = 4 images x 2 row-halves (64 rows each + halos); s=1 cores
get vertically flipped images + dy-flipped conv weights (identical SPMD
program). kv partial sums AllReduced pairwise ([128,288] f32).

v2 vs baseline: all intermediates SBUF-resident (no ms/att/attf/h1 DRAM
round-trips), dw5+grouped-pw fused into block-diagonal PE matmuls for tiles
0-1 (DVE for tile 2), per-row DMA transposes replaced by PE transposes + Pool
evacuation, attention+projection fused per 4-row chunk, dw3 split across
PE/DVE/Pool, bf16 output (host casts to f32).
"""
import contextlib

import numpy as np
import ml_dtypes

import concourse.bass as bass
import concourse.bacc as bacc
import concourse.tile as tile
from concourse import mybir
from concourse import bass_utils

F32 = mybir.dt.float32
BF16 = mybir.dt.bfloat16
Alu = mybir.AluOpType
AF = mybir.ActivationFunctionType
BF = ml_dtypes.bfloat16

BN_EPS = 1e-5
NCORES = 8

_CACHE = {}
DBG = False

ATT_SRCS = {0: (0, 1, 2), 1: (2, 3, 4), 2: (4, 5)}  # att9 tile a -> multi tiles

# ---- engine split knobs ----
# dw5: t0 + t1-rows<24 fused on PE; t1-rows>=24 + t2-rows<36 on DVE;
# t2-rows>=36 on Pool (see _head emission)
DW3_ENG = ("pe", "pe", "pe", "pe", "dve", "pe")  # per mid-tile t


def q_chan(g, e):
    return 24 * g + e if g < 16 else 384 + 24 * (g - 16) + e


def _chunks(lo, hi, step):
    out = []
    r = lo
    while r < hi:
        n = min(step, hi - r)
        out.append((r, n))
        r += n
    return out


def build_program():
    nc = bacc.Bacc("TRN2", target_bir_lowering=False, debug=False,
                   enable_asserts=False, num_devices=NCORES)
    d = {}
    def din(name, shape, dt):
        d[name] = nc.dram_tensor(name, shape, dt, kind="ExternalInput").ap()
    din("xr", [128, 72, 130], BF16)
    din("xo", [128, 72, 130], BF16)
    din("wc", [128, 27 * 128], BF16)
    din("cb", [128, 3], F32)
    din("cm", [128, 3], F32)
    din("wf", [128, 3 * 25 * 128], BF16)
    din("dw5", [128, 75], F32)
    din("pww", [128, 3 * 128], BF16)
    din("pjw", [128, 3 * 128], BF16)
    din("pjb", [128, 1], F32)
    din("m1w", [128, 6 * 128], BF16)
    din("m1b", [128, 6], F32)
    din("h1b", [128, 6], F32)
    din("dw3", [128, 54], F32)
    din("dw3d", [128, 54 * 128], BF16)
    din("m2b", [128, 6], F32)
    din("h2b", [128, 6], F32)
    din("m3w", [128, 6 * 128], BF16)
    din("m3b", [128, 1], F32)
    din("rb", [128, 66, 128], BF16)
    din("idn", [128, 128], BF16)
    din("eb", [12, 96], BF16)
    d["out"] = nc.dram_tensor("out", [128, 64, 128], BF16,
                              kind="ExternalOutput").ap()
    if DBG:
        for nm, sh, dt in (("dqkv", [3, 128, 70, 132], BF16),
                           ("dms", [3, 128, 66, 128], BF16),
                           ("dcomp", [128, 288], F32),
                           ("dcompR", [128, 288], F32),
                           ("dattf", [128, 66 * 128], BF16),
                           ("dh1", [6, 128, 66, 130], BF16)):
            d[nm] = nc.dram_tensor(nm, sh, dt, kind="ExternalOutput").ap()
    with tile.TileContext(nc) as tc:
        _emit(nc, tc, d)
    nc.compile()
    return nc


def _emit(nc, tc, d):
    env = {}
    with contextlib.ExitStack() as ctx:
        wp = ctx.enter_context(tc.tile_pool(name="wp", bufs=1))
        dram = ctx.enter_context(tc.tile_pool(name="dram", bufs=1, space="DRAM"))
        env["wp"] = wp

        def wtile(name, shape, dt, pool=wp):
            t = pool.tile(shape, dt, tag=name, name=name)
            nc.sync.dma_start(out=t, in_=d[name])
            return t
        env["wtile"] = wtile

        for nm, sh, dt in (("pjw", [128, 3 * 128], BF16), ("pjb", [128, 1], F32),
                           ("m1w", [128, 6 * 128], BF16), ("m1b", [128, 6], F32),
                           ("h1b", [128, 6], F32), ("dw3", [128, 54], F32),
                           ("m2b", [128, 6], F32), ("h2b", [128, 6], F32),
                           ("m3w", [128, 6 * 128], BF16), ("m3b", [128, 1], F32)):
            env[nm] = wtile(nm, sh, dt)
        env["attf"] = wp.tile([128, 66 * 128], BF16, tag="attf", name="attf")
        env["half"] = wp.tile([128, 1], F32, tag="half", name="half")
        nc.vector.memset(env["half"], 0.5)
        env["epsv"] = wp.tile([128, 1], F32, tag="epsv", name="epsv")
        nc.vector.memset(env["epsv"], 1e-6)
        env["comp"] = wp.tile([128, 288], F32, tag="comp", name="comp")
        env["compR"] = wp.tile([128, 288], F32, tag="compR", name="compR")
        env["cc_in"] = dram.tile([128, 288], F32, tag="cc_in", name="cc_in")
        env["cc_out"] = dram.tile([128, 288], F32, tag="cc_out", name="cc_out")
        env["rscr"] = dram.tile([12, 1536], BF16, tag="rscr", name="rscr")

        with tc.tile_pool(name="qkvms", bufs=1) as qp:
            env["qkv"] = [qp.tile([128, 70, 132], BF16, tag=f"qkv{t}",
                                  name=f"qkv{t}") for t in range(3)]
            env["ms"] = [qp.tile([128, 66, 128], BF16, tag=f"ms{t}",
                                 name=f"ms{t}") for t in range(3)]
            with contextlib.ExitStack() as hctx:
                _head(nc, tc, d, env, hctx)
            with contextlib.ExitStack() as mctx:
                _mid(nc, tc, d, env, mctx)
        with tc.tile_pool(name="tailp", bufs=1) as tp:
            env["h1"] = [tp.tile([128, 66, 130], BF16, tag=f"h1_{t}",
                                 name=f"h1_{t}") for t in range(6)]
            env["dw3d"] = wtile("dw3d", [128, 54 * 128], BF16, pool=tp)
            with contextlib.ExitStack() as tctx:
                _tail(nc, tc, d, env, tctx)


def _head(nc, tc, d, env, ctx):
    """conv qkv + dw5/pw (ms) + per-row transposes + kv accumulation."""
    qkv, ms = env["qkv"], env["ms"]
    wtile = env["wtile"]

    xp = ctx.enter_context(tc.tile_pool(name="xp", bufs=1))
    wc = wtile("wc", [128, 27 * 128], BF16, pool=xp)
    xr = xp.tile([128, 72, 130], BF16, tag="xr", name="xr")
    xo = xp.tile([128, 72, 130], BF16, tag="xo", name="xo")
    for (r0, rn) in ((0, 28), (28, 22), (50, 22)):
        nc.sync.dma_start(out=xr[:, r0:r0 + rn, :], in_=d["xr"][:, r0:r0 + rn, :])
        nc.sync.dma_start(out=xo[:, r0:r0 + rn, :], in_=d["xo"][:, r0:r0 + rn, :])
    wf = wtile("wf", [128, 3 * 25 * 128], BF16, pool=xp)
    cb = wtile("cb", [128, 3], F32, pool=xp)
    cm = wtile("cm", [128, 3], F32, pool=xp)
    dw5 = wtile("dw5", [128, 75], F32, pool=xp)
    pww = wtile("pww", [128, 3 * 128], BF16, pool=xp)
    idn = wtile("idn", [128, 128], BF16, pool=xp)
    ones = xp.tile([128, 1], BF16, tag="ones", name="ones")
    nc.vector.memset(ones, 1.0)
    env["cm"] = cm

    # pad memsets (cols 0:2 / 130:132 and top rows 0:3 of each qkv tile)
    for t in range(3):
        nc.vector.memset(qkv[t][:, :, 0:2], 0.0)
        nc.vector.memset(qkv[t][:, :, 130:132], 0.0)
        nc.vector.memset(qkv[t][:, 0:3, :], 0.0)

    cps = ctx.enter_context(tc.tile_pool(name="cps", bufs=2, space="PSUM"))
    fps = ctx.enter_context(tc.tile_pool(name="fps", bufs=2, space="PSUM"))
    tps = ctx.enter_context(tc.tile_pool(name="tps", bufs=3, space="PSUM"))
    kps = ctx.enter_context(tc.tile_pool(name="kps", bufs=1, space="PSUM"))
    mtp = ctx.enter_context(tc.tile_pool(name="mtp", bufs=3))
    dtp = ctx.enter_context(tc.tile_pool(name="dtp", bufs=1))
    app = ctx.enter_context(tc.tile_pool(name="app", bufs=1))

    kvps = kps.tile([128, 288], F32, tag="kvps", name="kvps")

    conv_chunks = _chunks(3, 70, 4)            # qkv tile rows (17)
    dw_chunks = _chunks(0, 66, 4)              # ms rows (17)

    def emit_conv(t, chunks):
        src = xr if t == 0 else xo
        for (r0, rn) in chunks:
            ps = cps.tile([128, 512], F32, tag="cps", name="cps")
            psv = ps[:, 0:rn * 128].rearrange("p (r w) -> p r w", w=128)
            first = True
            for dy in range(3):
                for dx in range(3):
                    k = (t * 9 + dy * 3 + dx) * 128
                    nc.tensor.matmul(
                        psv, wc[:, k:k + 128],
                        src[:, r0 + dy:r0 + dy + rn, dx:dx + 128],
                        start=first, stop=(dy == 2 and dx == 2))
                    first = False
            nc.scalar.activation(
                out=qkv[t][:, r0:r0 + rn, 2:130], in_=psv,
                func=AF.Identity, bias=cb[:, t:t + 1], scale=1.0)

    def emit_dwf(t, chunks):
        # fused dw5x5 + grouped-pw on PE: 25 block-diag matmuls per chunk
        for (c0, cn) in chunks:
            ps = fps.tile([128, 512], F32, tag="fps", name="fps")
            psv = ps[:, 0:cn * 128].rearrange("p (r w) -> p r w", w=128)
            for tap in range(25):
                dy, dx = tap // 5, tap % 5
                k = (t * 25 + tap) * 128
                nc.tensor.matmul(
                    psv, wf[:, k:k + 128],
                    qkv[t][:, c0 + dy:c0 + dy + cn, dx:dx + 128],
                    start=(tap == 0), stop=(tap == 24))
            nc.scalar.activation(out=ms[t][:, c0:c0 + cn, :], in_=psv,
                                 func=AF.Copy)
            nc.vector.tensor_scalar_max(
                out=ms[t][:, c0:c0 + cn, :], in0=ms[t][:, c0:c0 + cn, :],
                scalar1=cm[:, t:t + 1])

    def emit_dw5_dve(t, o0, o1):
        # dw5 via TS-mul + TT-add on DVE, acc in-place in ms[t]
        for (r0, n) in _chunks(o0, o1, 22):
            acc = ms[t][:, r0:r0 + n, :]
            for tap in range(25):
                dy, dx = tap // 5, tap % 5
                w_ap = dw5[:, t * 25 + tap:t * 25 + tap + 1]
                win = qkv[t][:, r0 + dy:r0 + dy + n, dx:dx + 128]
                if tap == 0:
                    nc.vector.tensor_scalar_mul(out=acc, in0=win, scalar1=w_ap)
                else:
                    tmp = dtp.tile([128, 22, 128], BF16, tag="dwtmp",
                                   name="dwtmp")
                    nc.vector.tensor_scalar_mul(out=tmp[:, 0:n, :], in0=win,
                                                scalar1=w_ap)
                    nc.vector.tensor_add(out=acc, in0=acc, in1=tmp[:, 0:n, :])

    def emit_dw5_ap(t, o0, o1, taps):
        # dw5 with multiplies on DVE (cheap 4x mode) and adds on Pool
        for (r0, n) in _chunks(o0, o1, 22):
            acc = ms[t][:, r0:r0 + n, :]
            for tap in taps:
                dy, dx = tap // 5, tap % 5
                w_ap = dw5[:, t * 25 + tap:t * 25 + tap + 1]
                win = qkv[t][:, r0 + dy:r0 + dy + n, dx:dx + 128]
                if tap == 0:
                    nc.vector.tensor_scalar_mul(out=acc, in0=win, scalar1=w_ap)
                else:
                    tmp = app.tile([128, 22, 128], BF16, tag="aptmp",
                                   name="aptmp")
                    nc.vector.tensor_scalar_mul(out=tmp[:, 0:n, :], in0=win,
                                                scalar1=w_ap)
                    nc.gpsimd.tensor_add(out=acc, in0=acc, in1=tmp[:, 0:n, :])

    def emit_pw(t, chunks):
        # grouped pw over raw dw5 acc already in ms[t]; evac back in place
        for (c0, cn) in chunks:
            ps = fps.tile([128, 512], F32, tag="fps", name="fps")
            psv = ps[:, 0:cn * 128].rearrange("p (r w) -> p r w", w=128)
            nc.tensor.matmul(psv, pww[:, t * 128:t * 128 + 128],
                             ms[t][:, c0:c0 + cn, :], start=True, stop=True)
            nc.scalar.activation(out=ms[t][:, c0:c0 + cn, :], in_=psv,
                                 func=AF.Copy)
            nc.vector.tensor_scalar_max(
                out=ms[t][:, c0:c0 + cn, :], in0=ms[t][:, c0:c0 + cn, :],
                scalar1=cm[:, t:t + 1])

    def emit_mask_qkv(t, r0, r1):
        nc.vector.tensor_scalar_max(out=qkv[t][:, r0:r1, :],
                                    in0=qkv[t][:, r0:r1, :],
                                    scalar1=cm[:, t:t + 1])

    def emit_kv(r0, r1):
        for r in range(r0, r1):
            tp_ = tps.tile([128, 768], BF16, tag="tp", name="tp")
            for t in range(3):
                nc.tensor.transpose(tp_[:, 128 * t:128 * t + 128],
                                    qkv[t][:, 3 + r, 2:130], idn)
                nc.tensor.transpose(tp_[:, 384 + 128 * t:384 + 128 * t + 128],
                                    ms[t][:, 1 + r, :], idn)
            mT = mtp.tile([128, 768], BF16, tag="mT", name="mT")
            # Pool/GPSIMD cannot read PSUM; DVE is saturated until the last
            # kv rows, so split the psum evacuation Act/DVE by row
            if r < 48:
                nc.scalar.activation(out=mT, in_=tp_, func=AF.Copy)
            else:
                nc.vector.tensor_copy(out=mT, in_=tp_)
            mg = mT.rearrange("p (g c) -> p g c", c=24)
            # matmul stationary operand needs a single free dim: gather k cols
            kc = mtp.tile([128, 256], BF16, tag="kc", name="kc")
            nc.gpsimd.tensor_copy(out=kc.rearrange("p (g e) -> p g e", e=8),
                                  in_=mg[:, :, 8:16])
            for h in range(2):
                kcols = kc[:, 128 * h:128 * h + 128]
                vcols = mg[:, 16 * h:16 * h + 16, 16:24]
                nc.tensor.matmul(kvps[:, 128 * h:128 * h + 128], kcols, vcols,
                                 start=(r == 0), stop=(r == 63),
                                 skip_group_check=True)
                nc.tensor.matmul(kvps[:, 256 + h:257 + h], kcols, ones,
                                 start=(r == 0), stop=(r == 63),
                                 skip_group_check=True)

    # ---- interleaved emission (PE stream must never block on DVE work
    # that gates only LATER PE items) ----
    for k in range(3):                       # conv rows 3..50 (12 chunks)
        for t in (2, 1, 0):
            emit_conv(t, conv_chunks[4 * k:4 * k + 4])
    emit_dwf(0, dw_chunks[0:6])
    emit_dwf(1, dw_chunks[0:6])
    emit_dw5_dve(2, 0, 22)
    emit_dwf(0, dw_chunks[6:11])
    emit_dw5_dve(2, 22, 44)
    emit_dw5_dve(1, 24, 44)
    for k in range(3, 5):                    # conv rows 51..69
        for t in (2, 1, 0):
            emit_conv(t, conv_chunks[4 * k:4 * k + 4])
    emit_pw(2, dw_chunks[0:11])
    emit_dwf(0, dw_chunks[11:17])
    emit_pw(1, dw_chunks[6:11])
    for t in range(3):
        emit_mask_qkv(t, 0, 36)
    emit_kv(0, 33)                           # kv row r needs qkv row 3+r, ms 1+r
    emit_dw5_dve(1, 44, 52)
    emit_dw5_dve(2, 44, 52)
    emit_dwf(2, dw_chunks[13:17])
    emit_dwf(1, dw_chunks[13:17])
    emit_pw(1, dw_chunks[11:13])
    emit_pw(2, dw_chunks[11:13])
    for t in range(3):
        emit_mask_qkv(t, 36, 70)
    emit_kv(33, 64)

    # comp extract: [128, 288] = 2 h-blocks of (16 g x 9): col 144h + 9*gl + dd
    comp = env["comp"]
    compv = comp.rearrange("p (h g e) -> p h g e", h=2, e=9)
    kvv = kvps[:, 0:256].rearrange("p (h g e) -> p h g e", h=2, e=8)
    for h in range(2):
        nc.scalar.activation(out=compv[:, h, :, 0:8], in_=kvv[:, h, :, :],
                             func=AF.Copy)
        ksrc = bass.AP(tensor=kvps.tensor, offset=kvps.offset + 256 + h,
                       ap=[[kvps.ap[0][0], 128], [0, 16], [1, 1]])
        nc.scalar.activation(out=compv[:, h, :, 8:9], in_=ksrc, func=AF.Copy)


def _mid(nc, tc, d, env, ctx):
    """AllReduce + scatter + fused att9 + proj + residual -> attf."""
    qkv, ms, attf = env["qkv"], env["ms"], env["attf"]
    pjw, pjb = env["pjw"], env["pjb"]
    comp, compR, wp = env["comp"], env["compR"], env["wp"]
    cc_in, cc_out, rscr = env["cc_in"], env["cc_out"], env["rscr"]

    if DBG:
        nc.sync.dma_start(out=d["dcomp"], in_=comp)
    nc.sync.dma_start(out=cc_in[:], in_=comp)
    nc.gpsimd.collective_compute(
        "AllReduce", Alu.add,
        replica_groups=[[0, 1], [2, 3], [4, 5], [6, 7]],
        ins=[cc_in.opt()], outs=[cc_out.opt()])
    nc.sync.dma_start(out=compR, in_=cc_out[:])
    if DBG:
        nc.sync.dma_start(out=d["dcompR"], in_=compR)

    lsp = ctx.enter_context(tc.tile_pool(name="lsp", bufs=1))
    lhsT_att = {}
    for a in ATT_SRCS:
        for S in ATT_SRCS[a]:
            st = lsp.tile([128, 108], F32, tag=f"lst{a}_{S}", name=f"lst{a}_{S}")
            nc.gpsimd.memset(st, 0.0)
            bt = lsp.tile([128, 108], BF16, tag=f"lat{a}_{S}", name=f"lat{a}_{S}")
            lhsT_att[(a, S)] = [st, bt]
    for g in range(32):
        a, gl9 = g // 12, g % 12
        h, gl = g // 16, g % 16
        S, row0 = q_chan(g, 0) // 128, q_chan(g, 0) % 128
        st = lhsT_att[(a, S)][0]
        L = st.rearrange("p (dd gl) -> p dd gl", gl=12)
        nc.sync.dma_start(
            out=L[row0:row0 + 8, 0:9, gl9:gl9 + 1],
            in_=compR[8 * gl:8 * gl + 8, 144 * h + 9 * gl:144 * h + 9 * gl + 9])
    for key, (st, bt) in lhsT_att.items():
        nc.vector.tensor_copy(out=bt, in_=st)
        lhsT_att[key] = bt

    eb = env["wtile"]("eb", [12, 96], BF16)

    aps = ctx.enter_context(tc.tile_pool(name="aps", bufs=6, space="PSUM"))
    jps = ctx.enter_context(tc.tile_pool(name="jps", bufs=1, space="PSUM"))
    dxp = ctx.enter_context(tc.tile_pool(name="dxp", bufs=1, space="PSUM"))
    dnp = ctx.enter_context(tc.tile_pool(name="dnp", bufs=3))
    arp = ctx.enter_context(tc.tile_pool(name="arp", bufs=4))

    for (c0, cn) in _chunks(0, 66, 4):
        cw = cn * 128

        def att_rhs(S):
            if S < 3:
                return qkv[S][:, 2 + c0:2 + c0 + cn, 2:130]
            return ms[S - 3][:, c0:c0 + cn, :]

        psl = []
        for a in range(3):
            ps = aps.tile([108, 512], F32, tag="aps", name="aps")
            srcs = ATT_SRCS[a]
            for i, S in enumerate(srcs):
                nc.tensor.matmul(ps[:, 0:cw], lhsT_att[(a, S)], att_rhs(S),
                                 start=(i == 0), stop=(i == len(srcs) - 1))
            psl.append(ps)
        den = dnp.tile([12, 3, 512], F32, tag="den", name="den")
        for a in range(3):
            # +eps so padding groups (den == 0) divide to 0, not NaN
            nc.scalar.activation(out=den[0:12, a, 0:cw],
                                 in_=psl[a][96:108, 0:cw], func=AF.Identity,
                                 bias=env["epsv"][0:12, 0:1], scale=1.0)
        rec = dnp.tile([12, 3, 512], BF16, tag="rec", name="rec")
        with nc.allow_low_precision(reason="den recip to bf16"):
            nc.vector.reciprocal(out=rec[:, :, 0:cw], in_=den[:, :, 0:cw])
        jp = jps.tile([128, 512], F32, tag="jps", name="jps")
        dexp = dnp.tile([96, 3, 512], BF16, tag="dexp", name="dexp")
        for a in range(3):
            # broadcast 12 group-divisors to 96 (dd,gl) rows via 0/1 matmul
            dxt = dxp.tile([96, 512], F32, tag="dxt", name="dxt")
            nc.tensor.matmul(dxt[:, 0:cw], eb[0:12, 0:96],
                             rec[:, a, 0:cw], start=True, stop=True)
            nc.scalar.activation(out=dexp[:, a, 0:cw], in_=dxt[:, 0:cw],
                                 func=AF.Copy)
            attc = arp.tile([96, 512], BF16, tag="attc", name="attc")
            nc.vector.tensor_mul(out=attc[:, 0:cw], in0=dexp[:, a, 0:cw],
                                 in1=psl[a][0:96, 0:cw])
            nc.tensor.matmul(jp[:, 0:cw], pjw[0:96, a * 128:a * 128 + 128],
                             attc[:, 0:cw], start=(a == 0), stop=(a == 2))
        attB = arp.tile([128, 512], BF16, tag="attB", name="attB")
        nc.scalar.activation(out=attB[:, 0:cw], in_=jp[:, 0:cw],
                             func=AF.Identity, bias=pjb[:, 0:1], scale=1.0)
        rbt = arp.tile([128, 4, 128], BF16, tag="rbt", name="rbt")
        nc.scalar.dma_start(out=rbt[:, 0:cn, :], in_=d["rb"][:, c0:c0 + cn, :])
        nc.gpsimd.tensor_add(
            out=attf[:, c0 * 128:c0 * 128 + cw], in0=attB[:, 0:cw],
            in1=rbt[:, 0:cn, :].rearrange("p r w -> p (r w)"))


def _tail(nc, tc, d, env, ctx):
    """mb1 + hswish -> h1 (SBUF); dw3 (PE/DVE/Pool) + hswish + mb3 + out."""
    attf = env["attf"]
    if DBG:
        nc.sync.dma_start(out=d["dattf"], in_=attf)
    m1w, m1b, h1b = env["m1w"], env["m1b"], env["h1b"]
    dw3, dw3d, m2b, h2b = env["dw3"], env["dw3d"], env["m2b"], env["h2b"]
    m3w, m3b = env["m3w"], env["m3b"]
    h1 = env["h1"]

    # shared scratch psum: mb1 and dw3-PE outputs round-robin one tag
    sps = ctx.enter_context(tc.tile_pool(name="sps", bufs=3, space="PSUM"))
    m3ps = ctx.enter_context(tc.tile_pool(name="m3ps", bufs=1, space="PSUM"))
    hwp = ctx.enter_context(tc.tile_pool(name="hwp", bufs=4))
    mulp = ctx.enter_context(tc.tile_pool(name="mulp", bufs=7))
    thp = ctx.enter_context(tc.tile_pool(name="thp", bufs=4))
    xap = ctx.enter_context(tc.tile_pool(name="xap", bufs=3))
    accp = ctx.enter_context(tc.tile_pool(name="accp", bufs=3))
    osp = ctx.enter_context(tc.tile_pool(name="osp", bufs=2))

    for t in range(6):
        nc.gpsimd.memset(h1[t][:, :, 0:1], 0.0)
        nc.gpsimd.memset(h1[t][:, :, 129:130], 0.0)
        nc.gpsimd.memset(h1[t][:, 0:1, :], 0.0)

    p8_chunks = _chunks(0, 66, 8)   # 9
    p9_chunks = _chunks(0, 64, 8)   # 8

    def emit_p8(j):
        c0, cn = p8_chunks[j]
        for t in range(6):
            ps = sps.tile([128, 1024], F32, tag="sps", name="sps")
            for (s0, sn) in _chunks(c0, c0 + cn, 4):
                nc.tensor.matmul(
                    ps[:, (s0 - c0) * 128:(s0 - c0 + sn) * 128],
                    m1w[:, t * 128:t * 128 + 128],
                    attf[:, s0 * 128:(s0 + sn) * 128],
                    start=True, stop=True)
            pw_ = ps[:, 0:cn * 128]
            th = hwp.tile([128, 1024], BF16, tag="th", name="th")
            nc.scalar.activation(out=th[:, 0:cn * 128], in_=pw_, func=AF.Relu,
                                 bias=h1b[:, t:t + 1], scale=1.0 / 6.0)
            nc.vector.tensor_scalar_min(out=th[:, 0:cn * 128],
                                        in0=th[:, 0:cn * 128], scalar1=1.0)
            r0 = max(c0, 1)   # keep h1 row 0 zero (dw3 top pad)
            off = (r0 - c0) * 128
            hout = h1[t][:, r0:c0 + cn, 1:129]
            thv = th[:, off:cn * 128].rearrange("p (r w) -> p r w", w=128)
            if t < 4:
                # xh on Act, multiply on Pool (SBUF-only engine)
                xh = hwp.tile([128, 1024], BF16, tag="xh", name="xh")
                nc.scalar.activation(out=xh[:, 0:cn * 128], in_=pw_,
                                     func=AF.Identity, bias=m1b[:, t:t + 1],
                                     scale=1.0)
                nc.gpsimd.tensor_mul(
                    out=hout, in0=thv,
                    in1=xh[:, off:cn * 128].rearrange("p (r w) -> p r w",
                                                      w=128))
            else:
                # fused (psum + m1b) * min(relu(.),1) on DVE
                pv = ps[:, off:cn * 128].rearrange("p (r w) -> p r w", w=128)
                nc.vector.scalar_tensor_tensor(
                    out=hout, in0=pv, scalar=m1b[:, t:t + 1], in1=thv,
                    op0=Alu.add, op1=Alu.mult)

    def emit_p9(j):
        q0, qn = p9_chunks[j]
        mp = m3ps.tile([128, 1024], F32, tag="m3ps", name="m3ps")
        h2fs = [None] * 6
        # pass 1: dw3 + hswish per tile (PE runs ahead across tiles); PE
        # tiles first so their hswish (DVE) unblocks mb3 early, DVE-dw3 last
        for t in (0, 1, 2, 3, 5, 4):
            eng = DW3_ENG[t]
            h2 = mulp.tile([128, 8, 128], BF16, tag="h2", name="h2")
            if eng == "pe":
                ps = sps.tile([128, 1024], F32, tag="sps", name="sps")
                psv = ps[:, 0:qn * 128].rearrange("p (r w) -> p r w", w=128)
                for (s0, sn) in _chunks(0, qn, 4):
                    for tap in range(9):
                        dy, dx = tap // 3, tap % 3
                        k = (t * 9 + tap) * 128
                        nc.tensor.matmul(
                            psv[:, s0:s0 + sn, :], dw3d[:, k:k + 128],
                            h1[t][:, q0 + s0 + dy:q0 + s0 + dy + sn,
                                  dx:dx + 128],
                            start=(tap == 0), stop=(tap == 8))
                th2 = thp.tile([128, 1024], BF16, tag="th2", name="th2")
                nc.scalar.activation(out=th2[:, 0:qn * 128],
                                     in_=ps[:, 0:qn * 128], func=AF.Relu,
                                     bias=h2b[:, t:t + 1], scale=1.0 / 6.0)
                if t < 2:
                    xa = xap.tile([128, 1024], BF16, tag="xa", name="xa")
                    nc.scalar.activation(out=xa[:, 0:qn * 128],
                                         in_=ps[:, 0:qn * 128],
                                         func=AF.Identity,
                                         bias=m2b[:, t:t + 1], scale=1.0)
                    accf = xa[:, 0:qn * 128]
                else:
                    accf = ps[:, 0:qn * 128]   # fused add in STT below
            else:
                acc = accp.tile([128, 8, 128], BF16, tag="acc", name="acc")
                av = acc[:, 0:qn, :]
                for tap in range(9):
                    dy, dx = tap // 3, tap % 3
                    w_ap = dw3[:, t * 9 + tap:t * 9 + tap + 1]
                    win = h1[t][:, q0 + dy:q0 + dy + qn, dx:dx + 128]
                    if eng == "dve":
                        if tap == 0:
                            nc.vector.tensor_scalar(
                                out=av, in0=win, scalar1=w_ap,
                                scalar2=m2b[:, t:t + 1], op0=Alu.mult,
                                op1=Alu.add)
                        else:
                            tmp = accp.tile([128, 8, 128], BF16, tag="tmp",
                                           name="tmp")
                            nc.vector.tensor_scalar_mul(
                                out=tmp[:, 0:qn, :], in0=win, scalar1=w_ap)
                            nc.vector.tensor_add(out=av, in0=av,
                                                 in1=tmp[:, 0:qn, :])
                    else:
                        if tap == 0:
                            nc.gpsimd.tensor_scalar(
                                out=av, in0=win, scalar1=w_ap,
                                scalar2=m2b[:, t:t + 1], op0=Alu.mult,
                                op1=Alu.add)
                        else:
                            nc.gpsimd.scalar_tensor_tensor(
                                out=av, in0=win, scalar=w_ap, in1=av,
                                op0=Alu.mult, op1=Alu.add)
                th2 = thp.tile([128, 1024], BF16, tag="th2", name="th2")
                accf = av.rearrange("p r w -> p (r w)")
                nc.scalar.activation(out=th2[:, 0:qn * 128], in_=accf,
                                     func=AF.Relu, bias=env["half"][:, 0:1],
                                     scale=1.0 / 6.0)
            nc.vector.tensor_scalar_min(out=th2[:, 0:qn * 128],
                                        in0=th2[:, 0:qn * 128], scalar1=1.0)
            h2f = h2.rearrange("p r w -> p (r w)")[:, 0:qn * 128]
            if eng == "pe" and t >= 2:
                nc.vector.scalar_tensor_tensor(
                    out=h2f, in0=accf, scalar=m2b[:, t:t + 1],
                    in1=th2[:, 0:qn * 128], op0=Alu.add, op1=Alu.mult)
            elif t < 2:
                nc.gpsimd.tensor_mul(out=h2f, in0=th2[:, 0:qn * 128],
                                     in1=accf)
            else:
                nc.vector.tensor_mul(out=h2f, in0=th2[:, 0:qn * 128],
                                     in1=accf)
            h2fs[t] = h2f
        # pass 2: mb3 accumulation (DVE-dw3 tile last — ready latest)
        p2 = (0, 1, 2, 3, 5, 4)
        for t in p2:
            for (s0, sn) in _chunks(0, qn, 4):
                nc.tensor.matmul(
                    mp[:, s0 * 128:(s0 + sn) * 128],
                    m3w[:, t * 128:t * 128 + 128],
                    h2fs[t][:, s0 * 128:(s0 + sn) * 128],
                    start=(t == p2[0]), stop=(t == p2[-1]))
        o1 = osp.tile([128, 1024], BF16, tag="o1", name="o1")
        nc.scalar.activation(out=o1[:, 0:qn * 128], in_=mp[:, 0:qn * 128],
                             func=AF.Identity, bias=m3b[:, 0:1], scale=1.0)
        nc.vector.tensor_add(
            out=o1[:, 0:qn * 128], in0=o1[:, 0:qn * 128],
            in1=attf[:, (q0 + 1) * 128:(q0 + 1 + qn) * 128])
        nc.sync.dma_start(
            out=d["out"][:, q0:q0 + qn, :],
            in_=o1[:, 0:qn * 128].rearrange("p (r w) -> p r w", w=128))

    emit_p8(0)
    for j in range(1, 9):
        emit_p8(j)
        emit_p9(j - 1)
    emit_p9(7)
    if DBG:
        for t in range(6):
            nc.sync.dma_start(out=d["dh1"][t], in_=h1[t])


# ====================== host side ======================

def _prep_shared(inp):
    f32 = np.float32
    out = {}
    pw = inp["agg_pw_w"][:, :, 0, 0]
    s1v = inp["bn1_g"] / np.sqrt(inp["bn1_v"] + BN_EPS)
    b1 = inp["bn1_b"] - inp["bn1_m"] * s1v
    Wp = inp["attn_proj_w"][:, :, 0, 0] * s1v[:, None]
    s2 = inp["bn2_g"] / np.sqrt(inp["bn2_v"] + BN_EPS)
    b2 = inp["bn2_b"] - inp["bn2_m"] * s2
    W3 = inp["mb3_w"][:, :, 0, 0] * s2[:, None]
    idn = np.eye(128, dtype=f32)

    for s in (0, 1):
        w = {}
        wc = np.zeros((128, 27 * 128), f32)
        for j, cw in enumerate((inp["wq"], inp["wk"], inp["wv"])):
            for dy in range(3):
                dyy = 2 - dy if s == 1 else dy
                for dx in range(3):
                    k = (j * 9 + dy * 3 + dx) * 128
                    wc[:, k:k + 128] = cw[:, :, dyy, dx].T
        w["wc"] = wc.astype(BF)
        w["cb"] = np.stack([inp["bq"], inp["bk"], inp["bv"]], 1).astype(f32)
        m = np.arange(384)
        w["cm"] = np.where((m % 24) < 16, 0.0, -1e9).astype(f32).reshape(3, 128).T.copy()
        dw5 = np.zeros((128, 75), f32)
        for t in range(3):
            for tap in range(25):
                dy, dx = tap // 5, tap % 5
                dyy = 4 - dy if s == 1 else dy
                dw5[:, t * 25 + tap] = inp["agg_dw_w"][128 * t:128 * t + 128, 0, dyy, dx]
        w["dw5"] = dw5
        # block-diag pw (per tile): pwbd[t][i, o] nonzero iff i//8 == o//8
        pwbd = np.zeros((3, 128, 128), f32)
        for mc in range(384):
            t, o = mc // 128, mc % 128
            g8 = (o // 8) * 8
            pwbd[t, g8:g8 + 8, o] = pw[mc]
        w["pww"] = pwbd.transpose(1, 0, 2).reshape(128, 384).astype(BF)
        # fused dw5+pw: wf[i, (t*25+tap)*128+o] = dw5[i,tap] * pwbd[t][i,o]
        wf = np.zeros((128, 3 * 25 * 128), f32)
        for t in range(3):
            for tap in range(25):
                k = (t * 25 + tap) * 128
                wf[:, k:k + 128] = pwbd[t] * dw5[:, t * 25 + tap][:, None]
        w["wf"] = wf.astype(BF)
        pjw = np.zeros((128, 3 * 128), f32)
        for g in range(32):
            a, gl9 = g // 12, g % 12
            for dd in range(8):
                pjw[12 * dd + gl9, a * 128:a * 128 + 128] = Wp[:, 8 * g + dd]
        w["pjw"] = pjw.astype(BF)
        w["pjb"] = b1.reshape(128, 1).astype(f32)
        m1w = np.zeros((128, 6 * 128), f32)
        for t in range(6):
            m1w[:, t * 128:t * 128 + 128] = inp["mb1_w"][128 * t:128 * t + 128, :, 0, 0].T
        w["m1w"] = m1w.astype(BF)
        w["m1b"] = inp["mb1_b"].reshape(6, 128).T.copy().astype(f32)
        w["h1b"] = (inp["mb1_b"].reshape(6, 128).T / 6.0 + 0.5).astype(f32)
        dw3 = np.zeros((128, 54), f32)
        for t in range(6):
            for tap in range(9):
                dy, dx = tap // 3, tap % 3
                dyy = 2 - dy if s == 1 else dy
                dw3[:, t * 9 + tap] = inp["mb2_w"][128 * t:128 * t + 128, 0, dyy, dx]
        w["dw3"] = dw3
        dw3d = np.zeros((128, 54 * 128), f32)
        for c in range(54):
            dw3d[:, c * 128:c * 128 + 128] = np.diag(dw3[:, c])
        w["dw3d"] = dw3d.astype(BF)
        w["m2b"] = inp["mb2_b"].reshape(6, 128).T.copy().astype(f32)
        w["h2b"] = (inp["mb2_b"].reshape(6, 128).T / 6.0 + 0.5).astype(f32)
        m3w = np.zeros((128, 6 * 128), f32)
        for t in range(6):
            m3w[:, t * 128:t * 128 + 128] = W3[:, 128 * t:128 * t + 128].T
        w["m3w"] = m3w.astype(BF)
        w["m3b"] = b2.reshape(128, 1).astype(f32)
        w["idn"] = idn.astype(BF)
        eb = np.zeros((12, 96), f32)
        for p in range(96):
            eb[p % 12, p] = 1.0
        w["eb"] = eb.astype(BF)
        out[s] = w
    return out


def _prep_core(inp, b, s):
    f32 = np.float32
    ref = inp["ref_features"][b]
    oth = inp["other_features"][b]
    if s == 1:
        ref = ref[:, ::-1, :]
        oth = oth[:, ::-1, :]
    xr = np.zeros((128, 72, 130), f32)
    xo = np.zeros((128, 72, 130), f32)
    xr[:, 4:72, 1:129] = ref[:, 0:68, :]
    xo[:, 4:72, 1:129] = oth[:, 0:68, :]
    rr = np.zeros((128, 66, 128), f32)
    rr[:, 1:66, :] = ref[:, 0:65, :]
    return {"xr": xr.astype(BF), "xo": xo.astype(BF), "rb": rr.astype(BF)}


def kernel(**inputs):
    inp = {k: np.asarray(v) for k, v in inputs.items()}
    if "nc" not in _CACHE:
        _CACHE["nc"] = build_program()
    nc = _CACHE["nc"]
    ws = _prep_shared(inp)
    in_maps = []
    for c in range(NCORES):
        b, s = c // 2, c % 2
        m = dict(ws[s])
        m.update(_prep_core(inp, b, s))
        in_maps.append(m)
    res = bass_utils.run_bass_kernel_spmd(nc, in_maps,
                                          core_ids=list(range(NCORES)))
    out = np.zeros((4, 128, 128, 128), np.float32)
    for c in range(NCORES):
        b, s = c // 2, c % 2
        o = res.results[c]["out"].astype(np.float32)
        if s == 1:
            o = o[:, ::-1, :]
        out[b, :, 64 * s:64 * s + 64, :] = o
    return out


# revision 6
# speedup vs baseline: 1.0714x; 1.0425x over previous
"""EfficientViT attention block on 8 TRN2 NeuronCores.

Sharding: 8 cores = 4 images x 2 row-halves (64 rows each + halos); s=1 cores
get vertically flipped images + dy-flipped conv weights (identical SPMD
program). kv partial sums AllReduced pairwise ([128,288] f32).

v2 vs baseline: all intermediates SBUF-resident (no ms/att/attf/h1 DRAM
round-trips), dw5+grouped-pw fused into block-diagonal PE matmuls for tiles
0-1 (DVE for tile 2), per-row DMA transposes replaced by PE transposes + Pool
evacuation, attention+projection fused per 4-row chunk, dw3 split across
PE/DVE/Pool, bf16 output (host casts to f32).
"""
import contextlib

import numpy as np
import ml_dtypes

import concourse.bass as bass
import concourse.bacc as bacc
import concourse.tile as tile
from concourse import mybir
from concourse import bass_utils

F32 = mybir.dt.float32
BF16 = mybir.dt.bfloat16
Alu = mybir.AluOpType
AF = mybir.ActivationFunctionType
BF = ml_dtypes.bfloat16

BN_EPS = 1e-5
NCORES = 8

_CACHE = {}
DBG = False

ATT_SRCS = {0: (0, 1, 2), 1: (2, 3, 4), 2: (4, 5)}  # att9 tile a -> multi tiles

# ---- engine split knobs ----
# dw5: t0 + t1-rows<24 fused on PE; t1-rows>=24 + t2-rows<36 on DVE;
# t2-rows>=36 on Pool (see _head emission)
DW3_ENG = ("pe", "pe", "pe", "pe", "dve", "pe")  # per mid-tile t


def q_chan(g, e):
    return 24 * g + e if g < 16 else 384 + 24 * (g - 16) + e


def _chunks(lo, hi, step):
    out = []
    r = lo
    while r < hi:
        n = min(step, hi - r)
        out.append((r, n))
        r += n
    return out


def build_program():
    nc = bacc.Bacc("TRN2", target_bir_lowering=False, debug=False,
                   enable_asserts=False, num_devices=NCORES)
    d = {}
    def din(name, shape, dt):
        d[name] = nc.dram_tensor(name, shape, dt, kind="ExternalInput").ap()
    din("xr", [128, 72, 130], BF16)
    din("xo", [128, 72, 130], BF16)
    din("wc", [128, 27 * 128], BF16)
    din("cb", [128, 3], F32)
    din("cm", [128, 3], F32)
    din("wf", [128, 3 * 25 * 128], BF16)
    din("dw5", [128, 75], F32)
    din("pww", [128, 3 * 128], BF16)
    din("pjw", [128, 3 * 128], BF16)
    din("pjb", [128, 1], F32)
    din("m1w", [128, 6 * 128], BF16)
    din("m1b", [128, 6], F32)
    din("h1b", [128, 6], F32)
    din("dw3", [128, 54], F32)
    din("dw3d", [128, 54 * 128], BF16)
    din("m2b", [128, 6], F32)
    din("h2b", [128, 6], F32)
    din("m3w", [128, 6 * 128], BF16)
    din("m3b", [128, 1], F32)
    din("rb", [128, 66, 128], BF16)
    din("idn", [128, 128], BF16)
    din("eb", [12, 96], BF16)
    d["out"] = nc.dram_tensor("out", [128, 64, 128], BF16,
                              kind="ExternalOutput").ap()
    if DBG:
        for nm, sh, dt in (("dqkv", [3, 128, 70, 132], BF16),
                           ("dms", [3, 128, 66, 128], BF16),
                           ("dcomp", [128, 288], F32),
                           ("dcompR", [128, 288], F32),
                           ("dattf", [128, 66 * 128], BF16),
                           ("dh1", [6, 128, 66, 130], BF16)):
            d[nm] = nc.dram_tensor(nm, sh, dt, kind="ExternalOutput").ap()
    with tile.TileContext(nc) as tc:
        _emit(nc, tc, d)
    nc.compile()
    return nc


def _emit(nc, tc, d):
    env = {}
    with contextlib.ExitStack() as ctx:
        wp = ctx.enter_context(tc.tile_pool(name="wp", bufs=1))
        dram = ctx.enter_context(tc.tile_pool(name="dram", bufs=1, space="DRAM"))
        env["wp"] = wp

        def wtile(name, shape, dt, pool=wp):
            t = pool.tile(shape, dt, tag=name, name=name)
            nc.sync.dma_start(out=t, in_=d[name])
            return t
        env["wtile"] = wtile

        for nm, sh, dt in (("pjw", [128, 3 * 128], BF16), ("pjb", [128, 1], F32),
                           ("m1w", [128, 6 * 128], BF16), ("m1b", [128, 6], F32),
                           ("h1b", [128, 6], F32), ("dw3", [128, 54], F32),
                           ("m2b", [128, 6], F32), ("h2b", [128, 6], F32),
                           ("m3w", [128, 6 * 128], BF16), ("m3b", [128, 1], F32)):
            env[nm] = wtile(nm, sh, dt)
        env["attf"] = wp.tile([128, 66 * 128], BF16, tag="attf", name="attf")
        env["half"] = wp.tile([128, 1], F32, tag="half", name="half")
        nc.vector.memset(env["half"], 0.5)
        env["epsv"] = wp.tile([128, 1], F32, tag="epsv", name="epsv")
        nc.vector.memset(env["epsv"], 1e-6)
        env["comp"] = wp.tile([128, 288], F32, tag="comp", name="comp")
        env["compR"] = wp.tile([128, 288], F32, tag="compR", name="compR")
        env["cc_in"] = dram.tile([128, 288], F32, tag="cc_in", name="cc_in")
        env["cc_out"] = dram.tile([128, 288], F32, tag="cc_out", name="cc_out")
        env["rscr"] = dram.tile([12, 1536], BF16, tag="rscr", name="rscr")

        with tc.tile_pool(name="qkvms", bufs=1) as qp:
            env["qkv"] = [qp.tile([128, 70, 132], BF16, tag=f"qkv{t}",
                                  name=f"qkv{t}") for t in range(3)]
            env["ms"] = [qp.tile([128, 66, 128], BF16, tag=f"ms{t}",
                                 name=f"ms{t}") for t in range(3)]
            with contextlib.ExitStack() as hctx:
                _head(nc, tc, d, env, hctx)
            with contextlib.ExitStack() as mctx:
                _mid(nc, tc, d, env, mctx)
        with tc.tile_pool(name="tailp", bufs=1) as tp:
            env["h1"] = [tp.tile([128, 66, 130], BF16, tag=f"h1_{t}",
                                 name=f"h1_{t}") for t in range(6)]
            env["dw3d"] = wtile("dw3d", [128, 54 * 128], BF16, pool=tp)
            with contextlib.ExitStack() as tctx:
                _tail(nc, tc, d, env, tctx)


def _head(nc, tc, d, env, ctx):
    """conv qkv + dw5/pw (ms) + per-row transposes + kv accumulation."""
    qkv, ms = env["qkv"], env["ms"]
    wtile = env["wtile"]

    xp = ctx.enter_context(tc.tile_pool(name="xp", bufs=1))
    wc = wtile("wc", [128, 27 * 128], BF16, pool=xp)
    xr = xp.tile([128, 72, 130], BF16, tag="xr", name="xr")
    xo = xp.tile([128, 72, 130], BF16, tag="xo", name="xo")
    for (r0, rn) in ((0, 28), (28, 22), (50, 22)):
        nc.sync.dma_start(out=xr[:, r0:r0 + rn, :], in_=d["xr"][:, r0:r0 + rn, :])
        nc.sync.dma_start(out=xo[:, r0:r0 + rn, :], in_=d["xo"][:, r0:r0 + rn, :])
    wf = wtile("wf", [128, 3 * 25 * 128], BF16, pool=xp)
    cb = wtile("cb", [128, 3], F32, pool=xp)
    cm = wtile("cm", [128, 3], F32, pool=xp)
    dw5 = wtile("dw5", [128, 75], F32, pool=xp)
    pww = wtile("pww", [128, 3 * 128], BF16, pool=xp)
    idn = wtile("idn", [128, 128], BF16, pool=xp)
    ones = xp.tile([128, 1], BF16, tag="ones", name="ones")
    nc.vector.memset(ones, 1.0)
    env["cm"] = cm

    # pad memsets (cols 0:2 / 130:132 and top rows 0:3 of each qkv tile)
    for t in range(3):
        nc.vector.memset(qkv[t][:, :, 0:2], 0.0)
        nc.vector.memset(qkv[t][:, :, 130:132], 0.0)
        nc.vector.memset(qkv[t][:, 0:3, :], 0.0)

    cps = ctx.enter_context(tc.tile_pool(name="cps", bufs=2, space="PSUM"))
    fps = ctx.enter_context(tc.tile_pool(name="fps", bufs=2, space="PSUM"))
    tps = ctx.enter_context(tc.tile_pool(name="tps", bufs=3, space="PSUM"))
    kps = ctx.enter_context(tc.tile_pool(name="kps", bufs=1, space="PSUM"))
    mtp = ctx.enter_context(tc.tile_pool(name="mtp", bufs=3))
    dtp = ctx.enter_context(tc.tile_pool(name="dtp", bufs=1))
    app = ctx.enter_context(tc.tile_pool(name="app", bufs=1))

    kvps = kps.tile([128, 288], F32, tag="kvps", name="kvps")

    conv_chunks = _chunks(3, 70, 4)            # qkv tile rows (17)
    dw_chunks = _chunks(0, 66, 4)              # ms rows (17)

    def emit_conv(t, chunks):
        src = xr if t == 0 else xo
        for (r0, rn) in chunks:
            ps = cps.tile([128, 512], F32, tag="cps", name="cps")
            psv = ps[:, 0:rn * 128].rearrange("p (r w) -> p r w", w=128)
            first = True
            for dy in range(3):
                for dx in range(3):
                    k = (t * 9 + dy * 3 + dx) * 128
                    nc.tensor.matmul(
                        psv, wc[:, k:k + 128],
                        src[:, r0 + dy:r0 + dy + rn, dx:dx + 128],
                        start=first, stop=(dy == 2 and dx == 2))
                    first = False
            nc.scalar.activation(
                out=qkv[t][:, r0:r0 + rn, 2:130], in_=psv,
                func=AF.Identity, bias=cb[:, t:t + 1], scale=1.0)

    def emit_dwf(t, chunks):
        # fused dw5x5 + grouped-pw on PE: 25 block-diag matmuls per chunk
        for (c0, cn) in chunks:
            ps = fps.tile([128, 512], F32, tag="fps", name="fps")
            psv = ps[:, 0:cn * 128].rearrange("p (r w) -> p r w", w=128)
            for tap in range(25):
                dy, dx = tap // 5, tap % 5
                k = (t * 25 + tap) * 128
                nc.tensor.matmul(
                    psv, wf[:, k:k + 128],
                    qkv[t][:, c0 + dy:c0 + dy + cn, dx:dx + 128],
                    start=(tap == 0), stop=(tap == 24))
            nc.scalar.activation(out=ms[t][:, c0:c0 + cn, :], in_=psv,
                                 func=AF.Copy)
            nc.vector.tensor_scalar_max(
                out=ms[t][:, c0:c0 + cn, :], in0=ms[t][:, c0:c0 + cn, :],
                scalar1=cm[:, t:t + 1])

    def emit_dw5_dve(t, o0, o1):
        # dw5 via TS-mul + TT-add on DVE, acc in-place in ms[t]
        for (r0, n) in _chunks(o0, o1, 22):
            acc = ms[t][:, r0:r0 + n, :]
            for tap in range(25):
                dy, dx = tap // 5, tap % 5
                w_ap = dw5[:, t * 25 + tap:t * 25 + tap + 1]
                win = qkv[t][:, r0 + dy:r0 + dy + n, dx:dx + 128]
                if tap == 0:
                    nc.vector.tensor_scalar_mul(out=acc, in0=win, scalar1=w_ap)
                else:
                    tmp = dtp.tile([128, 22, 128], BF16, tag="dwtmp",
                                   name="dwtmp")
                    nc.vector.tensor_scalar_mul(out=tmp[:, 0:n, :], in0=win,
                                                scalar1=w_ap)
                    nc.vector.tensor_add(out=acc, in0=acc, in1=tmp[:, 0:n, :])

    def emit_dw5_ap(t, o0, o1, taps):
        # dw5 with multiplies on DVE (cheap 4x mode) and adds on Pool
        for (r0, n) in _chunks(o0, o1, 22):
            acc = ms[t][:, r0:r0 + n, :]
            for tap in taps:
                dy, dx = tap // 5, tap % 5
                w_ap = dw5[:, t * 25 + tap:t * 25 + tap + 1]
                win = qkv[t][:, r0 + dy:r0 + dy + n, dx:dx + 128]
                if tap == 0:
                    nc.vector.tensor_scalar_mul(out=acc, in0=win, scalar1=w_ap)
                else:
                    tmp = app.tile([128, 22, 128], BF16, tag="aptmp",
                                   name="aptmp")
                    nc.vector.tensor_scalar_mul(out=tmp[:, 0:n, :], in0=win,
                                                scalar1=w_ap)
                    nc.gpsimd.tensor_add(out=acc, in0=acc, in1=tmp[:, 0:n, :])

    def emit_pw(t, chunks):
        # grouped pw over raw dw5 acc already in ms[t]; evac back in place
        for (c0, cn) in chunks:
            ps = fps.tile([128, 512], F32, tag="fps", name="fps")
            psv = ps[:, 0:cn * 128].rearrange("p (r w) -> p r w", w=128)
            nc.tensor.matmul(psv, pww[:, t * 128:t * 128 + 128],
                             ms[t][:, c0:c0 + cn, :], start=True, stop=True)
            nc.scalar.activation(out=ms[t][:, c0:c0 + cn, :], in_=psv,
                                 func=AF.Copy)
            nc.vector.tensor_scalar_max(
                out=ms[t][:, c0:c0 + cn, :], in0=ms[t][:, c0:c0 + cn, :],
                scalar1=cm[:, t:t + 1])

    def emit_mask_qkv(t, r0, r1):
        nc.vector.tensor_scalar_max(out=qkv[t][:, r0:r1, :],
                                    in0=qkv[t][:, r0:r1, :],
                                    scalar1=cm[:, t:t + 1])

    def emit_kv(r0, r1):
        for r in range(r0, r1):
            tp_ = tps.tile([128, 768], BF16, tag="tp", name="tp")
            for t in range(3):
                nc.tensor.transpose(tp_[:, 128 * t:128 * t + 128],
                                    qkv[t][:, 3 + r, 2:130], idn)
                nc.tensor.transpose(tp_[:, 384 + 128 * t:384 + 128 * t + 128],
                                    ms[t][:, 1 + r, :], idn)
            mT = mtp.tile([128, 768], BF16, tag="mT", name="mT")
            # Pool/GPSIMD cannot read PSUM; DVE is saturated until the last
            # kv rows, so split the psum evacuation Act/DVE by row
            if r < 48:
                nc.scalar.activation(out=mT, in_=tp_, func=AF.Copy)
            else:
                nc.vector.tensor_copy(out=mT, in_=tp_)
            mg = mT.rearrange("p (g c) -> p g c", c=24)
            # matmul stationary operand needs a single free dim: gather k cols
            kc = mtp.tile([128, 256], BF16, tag="kc", name="kc")
            nc.gpsimd.tensor_copy(out=kc.rearrange("p (g e) -> p g e", e=8),
                                  in_=mg[:, :, 8:16])
            for h in range(2):
                kcols = kc[:, 128 * h:128 * h + 128]
                vcols = mg[:, 16 * h:16 * h + 16, 16:24]
                nc.tensor.matmul(kvps[:, 128 * h:128 * h + 128], kcols, vcols,
                                 start=(r == 0), stop=(r == 63),
                                 skip_group_check=True)
                nc.tensor.matmul(kvps[:, 256 + h:257 + h], kcols, ones,
                                 start=(r == 0), stop=(r == 63),
                                 skip_group_check=True)

    # ---- interleaved emission (PE stream must never block on DVE work
    # that gates only LATER PE items) ----
    for k in range(3):                       # conv rows 3..50 (12 chunks)
        for t in (2, 1, 0):
            emit_conv(t, conv_chunks[4 * k:4 * k + 4])
    emit_dwf(0, dw_chunks[0:6])
    emit_dwf(1, dw_chunks[0:6])
    emit_dw5_dve(2, 0, 22)
    emit_dwf(0, dw_chunks[6:11])
    emit_dw5_dve(2, 22, 44)
    emit_dw5_dve(1, 24, 44)
    for k in range(3, 5):                    # conv rows 51..69
        for t in (2, 1, 0):
            emit_conv(t, conv_chunks[4 * k:4 * k + 4])
    emit_pw(2, dw_chunks[0:11])
    emit_dwf(0, dw_chunks[11:17])
    emit_pw(1, dw_chunks[6:11])
    for t in range(3):
        emit_mask_qkv(t, 0, 36)
    emit_kv(0, 33)                           # kv row r needs qkv row 3+r, ms 1+r
    emit_dw5_dve(1, 44, 52)
    emit_dw5_dve(2, 44, 52)
    emit_dwf(2, dw_chunks[13:17])
    emit_dwf(1, dw_chunks[13:17])
    emit_pw(1, dw_chunks[11:13])
    emit_pw(2, dw_chunks[11:13])
    for t in range(3):
        emit_mask_qkv(t, 36, 70)
    emit_kv(33, 64)

    # comp extract: [128, 288] = 2 h-blocks of (16 g x 9): col 144h + 9*gl + dd
    comp = env["comp"]
    compv = comp.rearrange("p (h g e) -> p h g e", h=2, e=9)
    kvv = kvps[:, 0:256].rearrange("p (h g e) -> p h g e", h=2, e=8)
    for h in range(2):
        nc.scalar.activation(out=compv[:, h, :, 0:8], in_=kvv[:, h, :, :],
                             func=AF.Copy)
        ksrc = bass.AP(tensor=kvps.tensor, offset=kvps.offset + 256 + h,
                       ap=[[kvps.ap[0][0], 128], [0, 16], [1, 1]])
        nc.scalar.activation(out=compv[:, h, :, 8:9], in_=ksrc, func=AF.Copy)


def _mid(nc, tc, d, env, ctx):
    """AllReduce + scatter + fused att9 + proj + residual -> attf."""
    qkv, ms, attf = env["qkv"], env["ms"], env["attf"]
    pjw, pjb = env["pjw"], env["pjb"]
    comp, compR, wp = env["comp"], env["compR"], env["wp"]
    cc_in, cc_out, rscr = env["cc_in"], env["cc_out"], env["rscr"]

    if DBG:
        nc.sync.dma_start(out=d["dcomp"], in_=comp)
    nc.sync.dma_start(out=cc_in[:], in_=comp)
    nc.gpsimd.collective_compute(
        "AllReduce", Alu.add,
        replica_groups=[[0, 1], [2, 3], [4, 5], [6, 7]],
        ins=[cc_in.opt()], outs=[cc_out.opt()])
    nc.sync.dma_start(out=compR, in_=cc_out[:])
    if DBG:
        nc.sync.dma_start(out=d["dcompR"], in_=compR)

    lsp = ctx.enter_context(tc.tile_pool(name="lsp", bufs=1))
    lhsT_att = {}
    for a in ATT_SRCS:
        for S in ATT_SRCS[a]:
            st = lsp.tile([128, 108], F32, tag=f"lst{a}_{S}", name=f"lst{a}_{S}")
            nc.gpsimd.memset(st, 0.0)
            bt = lsp.tile([128, 108], BF16, tag=f"lat{a}_{S}", name=f"lat{a}_{S}")
            lhsT_att[(a, S)] = [st, bt]
    for g in range(32):
        a, gl9 = g // 12, g % 12
        h, gl = g // 16, g % 16
        S, row0 = q_chan(g, 0) // 128, q_chan(g, 0) % 128
        st = lhsT_att[(a, S)][0]
        L = st.rearrange("p (dd gl) -> p dd gl", gl=12)
        nc.sync.dma_start(
            out=L[row0:row0 + 8, 0:9, gl9:gl9 + 1],
            in_=compR[8 * gl:8 * gl + 8, 144 * h + 9 * gl:144 * h + 9 * gl + 9])
    for key, (st, bt) in lhsT_att.items():
        nc.vector.tensor_copy(out=bt, in_=st)
        lhsT_att[key] = bt

    eb = env["wtile"]("eb", [12, 96], BF16)

    aps = ctx.enter_context(tc.tile_pool(name="aps", bufs=6, space="PSUM"))
    jps = ctx.enter_context(tc.tile_pool(name="jps", bufs=1, space="PSUM"))
    dxp = ctx.enter_context(tc.tile_pool(name="dxp", bufs=1, space="PSUM"))
    dnp = ctx.enter_context(tc.tile_pool(name="dnp", bufs=3))
    arp = ctx.enter_context(tc.tile_pool(name="arp", bufs=4))

    mchunks = _chunks(0, 66, 4)
    psl_all = {}

    def emit_att9(ci):
        c0, cn = mchunks[ci]
        cw = cn * 128

        def att_rhs(S):
            if S < 3:
                return qkv[S][:, 2 + c0:2 + c0 + cn, 2:130]
            return ms[S - 3][:, c0:c0 + cn, :]

        psl = []
        for a in range(3):
            ps = aps.tile([108, 512], F32, tag="aps", name="aps")
            srcs = ATT_SRCS[a]
            for i, S in enumerate(srcs):
                nc.tensor.matmul(ps[:, 0:cw], lhsT_att[(a, S)], att_rhs(S),
                                 start=(i == 0), stop=(i == len(srcs) - 1))
            psl.append(ps)
        psl_all[ci] = psl

    def emit_div(ci):
        c0, cn = mchunks[ci]
        cw = cn * 128
        psl = psl_all.pop(ci)
        den = dnp.tile([12, 3, 512], F32, tag="den", name="den")
        for a in range(3):
            # +eps so padding groups (den == 0) divide to 0, not NaN
            nc.scalar.activation(out=den[0:12, a, 0:cw],
                                 in_=psl[a][96:108, 0:cw], func=AF.Identity,
                                 bias=env["epsv"][0:12, 0:1], scale=1.0)
        rec = dnp.tile([12, 3, 512], BF16, tag="rec", name="rec")
        with nc.allow_low_precision(reason="den recip to bf16"):
            nc.vector.reciprocal(out=rec[:, :, 0:cw], in_=den[:, :, 0:cw])
        jp = jps.tile([128, 512], F32, tag="jps", name="jps")
        dexp = dnp.tile([96, 3, 512], BF16, tag="dexp", name="dexp")
        for a in range(3):
            # broadcast 12 group-divisors to 96 (dd,gl) rows via 0/1 matmul
            dxt = dxp.tile([96, 512], F32, tag="dxt", name="dxt")
            nc.tensor.matmul(dxt[:, 0:cw], eb[0:12, 0:96],
                             rec[:, a, 0:cw], start=True, stop=True)
            nc.scalar.activation(out=dexp[:, a, 0:cw], in_=dxt[:, 0:cw],
                                 func=AF.Copy)
            attc = arp.tile([96, 512], BF16, tag="attc", name="attc")
            nc.vector.tensor_mul(out=attc[:, 0:cw], in0=dexp[:, a, 0:cw],
                                 in1=psl[a][0:96, 0:cw])
            nc.tensor.matmul(jp[:, 0:cw], pjw[0:96, a * 128:a * 128 + 128],
                             attc[:, 0:cw], start=(a == 0), stop=(a == 2))
        attB = arp.tile([128, 512], BF16, tag="attB", name="attB")
        nc.scalar.activation(out=attB[:, 0:cw], in_=jp[:, 0:cw],
                             func=AF.Identity, bias=pjb[:, 0:1], scale=1.0)
        rbt = arp.tile([128, 4, 128], BF16, tag="rbt", name="rbt")
        nc.scalar.dma_start(out=rbt[:, 0:cn, :], in_=d["rb"][:, c0:c0 + cn, :])
        nc.gpsimd.tensor_add(
            out=attf[:, c0 * 128:c0 * 128 + cw], in0=attB[:, 0:cw],
            in1=rbt[:, 0:cn, :].rearrange("p r w -> p (r w)"))

    emit_att9(0)
    for ci in range(1, 17):
        emit_att9(ci)
        emit_div(ci - 1)
    emit_div(16)


def _tail(nc, tc, d, env, ctx):
    """mb1 + hswish -> h1 (SBUF); dw3 (PE/DVE/Pool) + hswish + mb3 + out."""
    attf = env["attf"]
    if DBG:
        nc.sync.dma_start(out=d["dattf"], in_=attf)
    m1w, m1b, h1b = env["m1w"], env["m1b"], env["h1b"]
    dw3, dw3d, m2b, h2b = env["dw3"], env["dw3d"], env["m2b"], env["h2b"]
    m3w, m3b = env["m3w"], env["m3b"]
    h1 = env["h1"]

    # shared scratch psum: mb1 and dw3-PE outputs round-robin one tag
    sps = ctx.enter_context(tc.tile_pool(name="sps", bufs=3, space="PSUM"))
    m3ps = ctx.enter_context(tc.tile_pool(name="m3ps", bufs=1, space="PSUM"))
    hwp = ctx.enter_context(tc.tile_pool(name="hwp", bufs=4))
    mulp = ctx.enter_context(tc.tile_pool(name="mulp", bufs=7))
    thp = ctx.enter_context(tc.tile_pool(name="thp", bufs=4))
    xap = ctx.enter_context(tc.tile_pool(name="xap", bufs=3))
    accp = ctx.enter_context(tc.tile_pool(name="accp", bufs=3))
    osp = ctx.enter_context(tc.tile_pool(name="osp", bufs=2))

    for t in range(6):
        nc.gpsimd.memset(h1[t][:, :, 0:1], 0.0)
        nc.gpsimd.memset(h1[t][:, :, 129:130], 0.0)
        nc.gpsimd.memset(h1[t][:, 0:1, :], 0.0)

    p8_chunks = _chunks(0, 66, 8)   # 9
    p9_chunks = _chunks(0, 64, 8)   # 8

    def emit_p8(j):
        c0, cn = p8_chunks[j]
        for t in range(6):
            ps = sps.tile([128, 1024], F32, tag="sps", name="sps")
            for (s0, sn) in _chunks(c0, c0 + cn, 4):
                nc.tensor.matmul(
                    ps[:, (s0 - c0) * 128:(s0 - c0 + sn) * 128],
                    m1w[:, t * 128:t * 128 + 128],
                    attf[:, s0 * 128:(s0 + sn) * 128],
                    start=True, stop=True)
            pw_ = ps[:, 0:cn * 128]
            th = hwp.tile([128, 1024], BF16, tag="th", name="th")
            nc.scalar.activation(out=th[:, 0:cn * 128], in_=pw_, func=AF.Relu,
                                 bias=h1b[:, t:t + 1], scale=1.0 / 6.0)
            nc.vector.tensor_scalar_min(out=th[:, 0:cn * 128],
                                        in0=th[:, 0:cn * 128], scalar1=1.0)
            r0 = max(c0, 1)   # keep h1 row 0 zero (dw3 top pad)
            off = (r0 - c0) * 128
            hout = h1[t][:, r0:c0 + cn, 1:129]
            thv = th[:, off:cn * 128].rearrange("p (r w) -> p r w", w=128)
            if t < 4:
                # xh on Act, multiply on Pool (SBUF-only engine)
                xh = hwp.tile([128, 1024], BF16, tag="xh", name="xh")
                nc.scalar.activation(out=xh[:, 0:cn * 128], in_=pw_,
                                     func=AF.Identity, bias=m1b[:, t:t + 1],
                                     scale=1.0)
                nc.gpsimd.tensor_mul(
                    out=hout, in0=thv,
                    in1=xh[:, off:cn * 128].rearrange("p (r w) -> p r w",
                                                      w=128))
            else:
                # fused (psum + m1b) * min(relu(.),1) on DVE
                pv = ps[:, off:cn * 128].rearrange("p (r w) -> p r w", w=128)
                nc.vector.scalar_tensor_tensor(
                    out=hout, in0=pv, scalar=m1b[:, t:t + 1], in1=thv,
                    op0=Alu.add, op1=Alu.mult)

    h2fs_all = {}

    def emit_p9p1(j):
        q0, qn = p9_chunks[j]
        h2fs = [None] * 6
        # pass 1: dw3 + hswish per tile (PE runs ahead across tiles); PE
        # tiles first so their hswish (DVE) unblocks mb3 early, DVE-dw3 last
        for t in (0, 1, 2, 3, 5, 4):
            eng = DW3_ENG[t]
            h2 = mulp.tile([128, 8, 128], BF16, tag="h2", name="h2")
            if eng == "pe":
                ps = sps.tile([128, 1024], F32, tag="sps", name="sps")
                psv = ps[:, 0:qn * 128].rearrange("p (r w) -> p r w", w=128)
                for (s0, sn) in _chunks(0, qn, 4):
                    for tap in range(9):
                        dy, dx = tap // 3, tap % 3
                        k = (t * 9 + tap) * 128
                        nc.tensor.matmul(
                            psv[:, s0:s0 + sn, :], dw3d[:, k:k + 128],
                            h1[t][:, q0 + s0 + dy:q0 + s0 + dy + sn,
                                  dx:dx + 128],
                            start=(tap == 0), stop=(tap == 8))
                th2 = thp.tile([128, 1024], BF16, tag="th2", name="th2")
                nc.scalar.activation(out=th2[:, 0:qn * 128],
                                     in_=ps[:, 0:qn * 128], func=AF.Relu,
                                     bias=h2b[:, t:t + 1], scale=1.0 / 6.0)
                if t < 2:
                    xa = xap.tile([128, 1024], BF16, tag="xa", name="xa")
                    nc.scalar.activation(out=xa[:, 0:qn * 128],
                                         in_=ps[:, 0:qn * 128],
                                         func=AF.Identity,
                                         bias=m2b[:, t:t + 1], scale=1.0)
                    accf = xa[:, 0:qn * 128]
                else:
                    accf = ps[:, 0:qn * 128]   # fused add in STT below
            else:
                acc = accp.tile([128, 8, 128], BF16, tag="acc", name="acc")
                av = acc[:, 0:qn, :]
                for tap in range(9):
                    dy, dx = tap // 3, tap % 3
                    w_ap = dw3[:, t * 9 + tap:t * 9 + tap + 1]
                    win = h1[t][:, q0 + dy:q0 + dy + qn, dx:dx + 128]
                    if eng == "dve":
                        if tap == 0:
                            nc.vector.tensor_scalar(
                                out=av, in0=win, scalar1=w_ap,
                                scalar2=m2b[:, t:t + 1], op0=Alu.mult,
                                op1=Alu.add)
                        else:
                            tmp = accp.tile([128, 8, 128], BF16, tag="tmp",
                                           name="tmp")
                            nc.vector.tensor_scalar_mul(
                                out=tmp[:, 0:qn, :], in0=win, scalar1=w_ap)
                            nc.vector.tensor_add(out=av, in0=av,
                                                 in1=tmp[:, 0:qn, :])
                    else:
                        if tap == 0:
                            nc.gpsimd.tensor_scalar(
                                out=av, in0=win, scalar1=w_ap,
                                scalar2=m2b[:, t:t + 1], op0=Alu.mult,
                                op1=Alu.add)
                        else:
                            nc.gpsimd.scalar_tensor_tensor(
                                out=av, in0=win, scalar=w_ap, in1=av,
                                op0=Alu.mult, op1=Alu.add)
                th2 = thp.tile([128, 1024], BF16, tag="th2", name="th2")
                accf = av.rearrange("p r w -> p (r w)")
                nc.scalar.activation(out=th2[:, 0:qn * 128], in_=accf,
                                     func=AF.Relu, bias=env["half"][:, 0:1],
                                     scale=1.0 / 6.0)
            nc.vector.tensor_scalar_min(out=th2[:, 0:qn * 128],
                                        in0=th2[:, 0:qn * 128], scalar1=1.0)
            h2f = h2.rearrange("p r w -> p (r w)")[:, 0:qn * 128]
            if eng == "pe" and t >= 2:
                nc.vector.scalar_tensor_tensor(
                    out=h2f, in0=accf, scalar=m2b[:, t:t + 1],
                    in1=th2[:, 0:qn * 128], op0=Alu.add, op1=Alu.mult)
            elif t < 2:
                nc.gpsimd.tensor_mul(out=h2f, in0=th2[:, 0:qn * 128],
                                     in1=accf)
            else:
                nc.vector.tensor_mul(out=h2f, in0=th2[:, 0:qn * 128],
                                     in1=accf)
            h2fs[t] = h2f
        h2fs_all[j] = h2fs

    def emit_p9p2(j):
        q0, qn = p9_chunks[j]
        h2fs = h2fs_all.pop(j)
        mp = m3ps.tile([128, 1024], F32, tag="m3ps", name="m3ps")
        # mb3 accumulation (DVE-dw3 tile last — ready latest)
        p2 = (0, 1, 2, 3, 5, 4)
        for t in p2:
            for (s0, sn) in _chunks(0, qn, 4):
                nc.tensor.matmul(
                    mp[:, s0 * 128:(s0 + sn) * 128],
                    m3w[:, t * 128:t * 128 + 128],
                    h2fs[t][:, s0 * 128:(s0 + sn) * 128],
                    start=(t == p2[0]), stop=(t == p2[-1]))
        o1 = osp.tile([128, 1024], BF16, tag="o1", name="o1")
        nc.scalar.activation(out=o1[:, 0:qn * 128], in_=mp[:, 0:qn * 128],
                             func=AF.Identity, bias=m3b[:, 0:1], scale=1.0)
        nc.vector.tensor_add(
            out=o1[:, 0:qn * 128], in0=o1[:, 0:qn * 128],
            in1=attf[:, (q0 + 1) * 128:(q0 + 1 + qn) * 128])
        nc.sync.dma_start(
            out=d["out"][:, q0:q0 + qn, :],
            in_=o1[:, 0:qn * 128].rearrange("p (r w) -> p r w", w=128))

    emit_p8(0)
    emit_p8(1)
    emit_p9p1(0)
    for j in range(2, 9):
        emit_p8(j)
        emit_p9p2(j - 2)
        emit_p9p1(j - 1)
    emit_p9p2(7)
    if DBG:
        for t in range(6):
            nc.sync.dma_start(out=d["dh1"][t], in_=h1[t])


# ====================== host side ======================

def _prep_shared(inp):
    f32 = np.float32
    out = {}
    pw = inp["agg_pw_w"][:, :, 0, 0]
    s1v = inp["bn1_g"] / np.sqrt(inp["bn1_v"] + BN_EPS)
    b1 = inp["bn1_b"] - inp["bn1_m"] * s1v
    Wp = inp["attn_proj_w"][:, :, 0, 0] * s1v[:, None]
    s2 = inp["bn2_g"] / np.sqrt(inp["bn2_v"] + BN_EPS)
    b2 = inp["bn2_b"] - inp["bn2_m"] * s2
    W3 = inp["mb3_w"][:, :, 0, 0] * s2[:, None]
    idn = np.eye(128, dtype=f32)

    for s in (0, 1):
        w = {}
        wc = np.zeros((128, 27 * 128), f32)
        for j, cw in enumerate((inp["wq"], inp["wk"], inp["wv"])):
            for dy in range(3):
                dyy = 2 - dy if s == 1 else dy
                for dx in range(3):
                    k = (j * 9 + dy * 3 + dx) * 128
                    wc[:, k:k + 128] = cw[:, :, dyy, dx].T
        w["wc"] = wc.astype(BF)
        w["cb"] = np.stack([inp["bq"], inp["bk"], inp["bv"]], 1).astype(f32)
        m = np.arange(384)
        w["cm"] = np.where((m % 24) < 16, 0.0, -1e9).astype(f32).reshape(3, 128).T.copy()
        dw5 = np.zeros((128, 75), f32)
        for t in range(3):
            for tap in range(25):
                dy, dx = tap // 5, tap % 5
                dyy = 4 - dy if s == 1 else dy
                dw5[:, t * 25 + tap] = inp["agg_dw_w"][128 * t:128 * t + 128, 0, dyy, dx]
        w["dw5"] = dw5
        # block-diag pw (per tile): pwbd[t][i, o] nonzero iff i//8 == o//8
        pwbd = np.zeros((3, 128, 128), f32)
        for mc in range(384):
            t, o = mc // 128, mc % 128
            g8 = (o // 8) * 8
            pwbd[t, g8:g8 + 8, o] = pw[mc]
        w["pww"] = pwbd.transpose(1, 0, 2).reshape(128, 384).astype(BF)
        # fused dw5+pw: wf[i, (t*25+tap)*128+o] = dw5[i,tap] * pwbd[t][i,o]
        wf = np.zeros((128, 3 * 25 * 128), f32)
        for t in range(3):
            for tap in range(25):
                k = (t * 25 + tap) * 128
                wf[:, k:k + 128] = pwbd[t] * dw5[:, t * 25 + tap][:, None]
        w["wf"] = wf.astype(BF)
        pjw = np.zeros((128, 3 * 128), f32)
        for g in range(32):
            a, gl9 = g // 12, g % 12
            for dd in range(8):
                pjw[12 * dd + gl9, a * 128:a * 128 + 128] = Wp[:, 8 * g + dd]
        w["pjw"] = pjw.astype(BF)
        w["pjb"] = b1.reshape(128, 1).astype(f32)
        m1w = np.zeros((128, 6 * 128), f32)
        for t in range(6):
            m1w[:, t * 128:t * 128 + 128] = inp["mb1_w"][128 * t:128 * t + 128, :, 0, 0].T
        w["m1w"] = m1w.astype(BF)
        w["m1b"] = inp["mb1_b"].reshape(6, 128).T.copy().astype(f32)
        w["h1b"] = (inp["mb1_b"].reshape(6, 128).T / 6.0 + 0.5).astype(f32)
        dw3 = np.zeros((128, 54), f32)
        for t in range(6):
            for tap in range(9):
                dy, dx = tap // 3, tap % 3
                dyy = 2 - dy if s == 1 else dy
                dw3[:, t * 9 + tap] = inp["mb2_w"][128 * t:128 * t + 128, 0, dyy, dx]
        w["dw3"] = dw3
        dw3d = np.zeros((128, 54 * 128), f32)
        for c in range(54):
            dw3d[:, c * 128:c * 128 + 128] = np.diag(dw3[:, c])
        w["dw3d"] = dw3d.astype(BF)
        w["m2b"] = inp["mb2_b"].reshape(6, 128).T.copy().astype(f32)
        w["h2b"] = (inp["mb2_b"].reshape(6, 128).T / 6.0 + 0.5).astype(f32)
        m3w = np.zeros((128, 6 * 128), f32)
        for t in range(6):
            m3w[:, t * 128:t * 128 + 128] = W3[:, 128 * t:128 * t + 128].T
        w["m3w"] = m3w.astype(BF)
        w["m3b"] = b2.reshape(128, 1).astype(f32)
        w["idn"] = idn.astype(BF)
        eb = np.zeros((12, 96), f32)
        for p in range(96):
            eb[p % 12, p] = 1.0
        w["eb"] = eb.astype(BF)
        out[s] = w
    return out


def _prep_core(inp, b, s):
    f32 = np.float32
    ref = inp["ref_features"][b]
    oth = inp["other_features"][b]
    if s == 1:
        ref = ref[:, ::-1, :]
        oth = oth[:, ::-1, :]
    xr = np.zeros((128, 72, 130), f32)
    xo = np.zeros((128, 72, 130), f32)
    xr[:, 4:72, 1:129] = ref[:, 0:68, :]
    xo[:, 4:72, 1:129] = oth[:, 0:68, :]
    rr = np.zeros((128, 66, 128), f32)
    rr[:, 1:66, :] = ref[:, 0:65, :]
    return {"xr": xr.astype(BF), "xo": xo.astype(BF), "rb": rr.astype(BF)}


def kernel(**inputs):
    inp = {k: np.asarray(v) for k, v in inputs.items()}
    if "nc" not in _CACHE:
        _CACHE["nc"] = build_program()
    nc = _CACHE["nc"]
    ws = _prep_shared(inp)
    in_maps = []
    for c in range(NCORES):
        b, s = c // 2, c % 2
        m = dict(ws[s])
        m.update(_prep_core(inp, b, s))
        in_maps.append(m)
    res = bass_utils.run_bass_kernel_spmd(nc, in_maps,
                                          core_ids=list(range(NCORES)))
    out = np.zeros((4, 128, 128, 128), np.float32)
    for c in range(NCORES):
        b, s = c // 2, c % 2
        o = res.results[c]["out"].astype(np.float32)
        if s == 1:
            o = o[:, ::-1, :]
        out[b, :, 64 * s:64 * s + 64, :] = o
    return out


# revision 7
# speedup vs baseline: 1.0813x; 1.0093x over previous
"""EfficientViT attention block on 8 TRN2 NeuronCores.

Sharding: 8 cores = 4 images x 2 row-halves (64 rows each + halos); s=1 cores
get vertically flipped images + dy-flipped conv weights (identical SPMD
program). kv partial sums AllReduced pairwise ([128,288] f32).

v2 vs baseline: all intermediates SBUF-resident (no ms/att/attf/h1 DRAM
round-trips), dw5+grouped-pw fused into block-diagonal PE matmuls for tiles
0-1 (DVE for tile 2), per-row DMA transposes replaced by PE transposes + Pool
evacuation, attention+projection fused per 4-row chunk, dw3 split across
PE/DVE/Pool, bf16 output (host casts to f32).
"""
import contextlib

import numpy as np
import ml_dtypes

import concourse.bass as bass
import concourse.bacc as bacc
import concourse.tile as tile
from concourse import mybir
from concourse import bass_utils

F32 = mybir.dt.float32
BF16 = mybir.dt.bfloat16
Alu = mybir.AluOpType
AF = mybir.ActivationFunctionType
BF = ml_dtypes.bfloat16

BN_EPS = 1e-5
NCORES = 8

_CACHE = {}
DBG = False

ATT_SRCS = {0: (0, 1, 2), 1: (2, 3, 4), 2: (4, 5)}  # att9 tile a -> multi tiles

# ---- engine split knobs ----
# dw5: t0 + t1-rows<24 fused on PE; t1-rows>=24 + t2-rows<36 on DVE;
# t2-rows>=36 on Pool (see _head emission)
DW3_ENG = ("pe", "pe", "pe", "pe", "dve", "pe")  # per mid-tile t


def q_chan(g, e):
    return 24 * g + e if g < 16 else 384 + 24 * (g - 16) + e


def _chunks(lo, hi, step):
    out = []
    r = lo
    while r < hi:
        n = min(step, hi - r)
        out.append((r, n))
        r += n
    return out


def build_program():
    nc = bacc.Bacc("TRN2", target_bir_lowering=False, debug=False,
                   enable_asserts=False, num_devices=NCORES)
    d = {}
    def din(name, shape, dt):
        d[name] = nc.dram_tensor(name, shape, dt, kind="ExternalInput").ap()
    din("xr", [128, 72, 130], BF16)
    din("xo", [128, 72, 130], BF16)
    din("wc", [128, 27 * 128], BF16)
    din("cb", [128, 3], F32)
    din("cm", [128, 3], F32)
    din("wf", [128, 3 * 25 * 128], BF16)
    din("dw5", [128, 75], F32)
    din("pww", [128, 3 * 128], BF16)
    din("pjw", [128, 3 * 128], BF16)
    din("pjb", [128, 1], F32)
    din("m1w", [128, 6 * 128], BF16)
    din("m1b", [128, 6], F32)
    din("h1b", [128, 6], F32)
    din("dw3", [128, 54], F32)
    din("dw3d", [128, 54 * 128], BF16)
    din("m2b", [128, 6], F32)
    din("h2b", [128, 6], F32)
    din("m3w", [128, 6 * 128], BF16)
    din("m3b", [128, 1], F32)
    din("rb", [128, 66, 128], BF16)
    din("idn", [128, 128], BF16)
    din("eb", [12, 96], BF16)
    d["out"] = nc.dram_tensor("out", [128, 64, 128], BF16,
                              kind="ExternalOutput").ap()
    if DBG:
        for nm, sh, dt in (("dqkv", [3, 128, 70, 132], BF16),
                           ("dms", [3, 128, 66, 128], BF16),
                           ("dcomp", [128, 288], F32),
                           ("dcompR", [128, 288], F32),
                           ("dattf", [128, 66 * 128], BF16),
                           ("dh1", [6, 128, 66, 130], BF16)):
            d[nm] = nc.dram_tensor(nm, sh, dt, kind="ExternalOutput").ap()
    with tile.TileContext(nc) as tc:
        _emit(nc, tc, d)
    nc.compile()
    return nc


def _emit(nc, tc, d):
    env = {}
    with contextlib.ExitStack() as ctx:
        wp = ctx.enter_context(tc.tile_pool(name="wp", bufs=1))
        dram = ctx.enter_context(tc.tile_pool(name="dram", bufs=1, space="DRAM"))
        env["wp"] = wp

        def wtile(name, shape, dt, pool=wp):
            t = pool.tile(shape, dt, tag=name, name=name)
            nc.sync.dma_start(out=t, in_=d[name])
            return t
        env["wtile"] = wtile

        for nm, sh, dt in (("pjw", [128, 3 * 128], BF16), ("pjb", [128, 1], F32),
                           ("m1w", [128, 6 * 128], BF16), ("m1b", [128, 6], F32),
                           ("h1b", [128, 6], F32), ("dw3", [128, 54], F32),
                           ("m2b", [128, 6], F32), ("h2b", [128, 6], F32),
                           ("m3w", [128, 6 * 128], BF16), ("m3b", [128, 1], F32)):
            env[nm] = wtile(nm, sh, dt)
        env["attf"] = wp.tile([128, 66 * 128], BF16, tag="attf", name="attf")
        env["half"] = wp.tile([128, 1], F32, tag="half", name="half")
        nc.vector.memset(env["half"], 0.5)
        env["epsv"] = wp.tile([128, 1], F32, tag="epsv", name="epsv")
        nc.vector.memset(env["epsv"], 1e-6)
        env["comp"] = wp.tile([128, 288], F32, tag="comp", name="comp")
        env["compR"] = wp.tile([128, 288], F32, tag="compR", name="compR")
        env["cc_in"] = dram.tile([128, 288], F32, tag="cc_in", name="cc_in")
        env["cc_out"] = dram.tile([128, 288], F32, tag="cc_out", name="cc_out")
        env["rscr"] = dram.tile([12, 1536], BF16, tag="rscr", name="rscr")

        with tc.tile_pool(name="qkvms", bufs=1) as qp:
            env["qkv"] = [qp.tile([128, 70, 132], BF16, tag=f"qkv{t}",
                                  name=f"qkv{t}") for t in range(3)]
            env["ms"] = [qp.tile([128, 66, 128], BF16, tag=f"ms{t}",
                                 name=f"ms{t}") for t in range(3)]
            with contextlib.ExitStack() as hctx:
                _head(nc, tc, d, env, hctx)
            with contextlib.ExitStack() as mctx:
                _mid(nc, tc, d, env, mctx)
        with tc.tile_pool(name="tailp", bufs=1) as tp:
            env["h1"] = [tp.tile([128, 66, 130], BF16, tag=f"h1_{t}",
                                 name=f"h1_{t}") for t in range(6)]
            env["dw3d"] = wtile("dw3d", [128, 54 * 128], BF16, pool=tp)
            with contextlib.ExitStack() as tctx:
                _tail(nc, tc, d, env, tctx)


def _head(nc, tc, d, env, ctx):
    """conv qkv + dw5/pw (ms) + per-row transposes + kv accumulation."""
    qkv, ms = env["qkv"], env["ms"]
    wtile = env["wtile"]

    xp = ctx.enter_context(tc.tile_pool(name="xp", bufs=1))
    wc = wtile("wc", [128, 27 * 128], BF16, pool=xp)
    xr = xp.tile([128, 72, 130], BF16, tag="xr", name="xr")
    xo = xp.tile([128, 72, 130], BF16, tag="xo", name="xo")
    for (r0, rn) in ((0, 28), (28, 22), (50, 22)):
        nc.sync.dma_start(out=xr[:, r0:r0 + rn, :], in_=d["xr"][:, r0:r0 + rn, :])
        nc.sync.dma_start(out=xo[:, r0:r0 + rn, :], in_=d["xo"][:, r0:r0 + rn, :])
    wf = wtile("wf", [128, 3 * 25 * 128], BF16, pool=xp)
    cb = wtile("cb", [128, 3], F32, pool=xp)
    cm = wtile("cm", [128, 3], F32, pool=xp)
    dw5 = wtile("dw5", [128, 75], F32, pool=xp)
    pww = wtile("pww", [128, 3 * 128], BF16, pool=xp)
    idn = wtile("idn", [128, 128], BF16, pool=xp)
    ones = xp.tile([128, 1], BF16, tag="ones", name="ones")
    nc.vector.memset(ones, 1.0)
    env["cm"] = cm

    # pad memsets (cols 0:2 / 130:132 and top rows 0:3 of each qkv tile)
    for t in range(3):
        nc.vector.memset(qkv[t][:, :, 0:2], 0.0)
        nc.vector.memset(qkv[t][:, :, 130:132], 0.0)
        nc.vector.memset(qkv[t][:, 0:3, :], 0.0)

    cps = ctx.enter_context(tc.tile_pool(name="cps", bufs=3, space="PSUM"))
    fps = ctx.enter_context(tc.tile_pool(name="fps", bufs=2, space="PSUM"))
    tps = ctx.enter_context(tc.tile_pool(name="tps", bufs=2, space="PSUM"))
    kps = ctx.enter_context(tc.tile_pool(name="kps", bufs=1, space="PSUM"))
    mtp = ctx.enter_context(tc.tile_pool(name="mtp", bufs=3))
    dtp = ctx.enter_context(tc.tile_pool(name="dtp", bufs=1))
    app = ctx.enter_context(tc.tile_pool(name="app", bufs=1))

    kvps = kps.tile([128, 288], F32, tag="kvps", name="kvps")

    conv_chunks = _chunks(3, 70, 4)            # qkv tile rows (17)
    dw_chunks = _chunks(0, 66, 4)              # ms rows (17)

    def emit_conv(t, chunks):
        src = xr if t == 0 else xo
        for (r0, rn) in chunks:
            ps = cps.tile([128, 512], F32, tag="cps", name="cps")
            psv = ps[:, 0:rn * 128].rearrange("p (r w) -> p r w", w=128)
            first = True
            for dy in range(3):
                for dx in range(3):
                    k = (t * 9 + dy * 3 + dx) * 128
                    nc.tensor.matmul(
                        psv, wc[:, k:k + 128],
                        src[:, r0 + dy:r0 + dy + rn, dx:dx + 128],
                        start=first, stop=(dy == 2 and dx == 2))
                    first = False
            nc.scalar.activation(
                out=qkv[t][:, r0:r0 + rn, 2:130], in_=psv,
                func=AF.Identity, bias=cb[:, t:t + 1], scale=1.0)

    def emit_dwf(t, chunks):
        # fused dw5x5 + grouped-pw on PE: 25 block-diag matmuls per chunk
        for (c0, cn) in chunks:
            ps = fps.tile([128, 512], F32, tag="fps", name="fps")
            psv = ps[:, 0:cn * 128].rearrange("p (r w) -> p r w", w=128)
            for tap in range(25):
                dy, dx = tap // 5, tap % 5
                k = (t * 25 + tap) * 128
                nc.tensor.matmul(
                    psv, wf[:, k:k + 128],
                    qkv[t][:, c0 + dy:c0 + dy + cn, dx:dx + 128],
                    start=(tap == 0), stop=(tap == 24))
            nc.scalar.activation(out=ms[t][:, c0:c0 + cn, :], in_=psv,
                                 func=AF.Copy)
            nc.vector.tensor_scalar_max(
                out=ms[t][:, c0:c0 + cn, :], in0=ms[t][:, c0:c0 + cn, :],
                scalar1=cm[:, t:t + 1])

    def emit_dw5_dve(t, o0, o1):
        # dw5 via TS-mul + TT-add on DVE, acc in-place in ms[t]
        for (r0, n) in _chunks(o0, o1, 22):
            acc = ms[t][:, r0:r0 + n, :]
            for tap in range(25):
                dy, dx = tap // 5, tap % 5
                w_ap = dw5[:, t * 25 + tap:t * 25 + tap + 1]
                win = qkv[t][:, r0 + dy:r0 + dy + n, dx:dx + 128]
                if tap == 0:
                    nc.vector.tensor_scalar_mul(out=acc, in0=win, scalar1=w_ap)
                else:
                    tmp = dtp.tile([128, 22, 128], BF16, tag="dwtmp",
                                   name="dwtmp")
                    nc.vector.tensor_scalar_mul(out=tmp[:, 0:n, :], in0=win,
                                                scalar1=w_ap)
                    nc.vector.tensor_add(out=acc, in0=acc, in1=tmp[:, 0:n, :])

    def emit_dw5_ap(t, o0, o1, taps):
        # dw5 with multiplies on DVE (cheap 4x mode) and adds on Pool
        for (r0, n) in _chunks(o0, o1, 22):
            acc = ms[t][:, r0:r0 + n, :]
            for tap in taps:
                dy, dx = tap // 5, tap % 5
                w_ap = dw5[:, t * 25 + tap:t * 25 + tap + 1]
                win = qkv[t][:, r0 + dy:r0 + dy + n, dx:dx + 128]
                if tap == 0:
                    nc.vector.tensor_scalar_mul(out=acc, in0=win, scalar1=w_ap)
                else:
                    tmp = app.tile([128, 22, 128], BF16, tag="aptmp",
                                   name="aptmp")
                    nc.vector.tensor_scalar_mul(out=tmp[:, 0:n, :], in0=win,
                                                scalar1=w_ap)
                    nc.gpsimd.tensor_add(out=acc, in0=acc, in1=tmp[:, 0:n, :])

    def emit_pw(t, chunks):
        # grouped pw over raw dw5 acc already in ms[t]; evac back in place
        for (c0, cn) in chunks:
            ps = fps.tile([128, 512], F32, tag="fps", name="fps")
            psv = ps[:, 0:cn * 128].rearrange("p (r w) -> p r w", w=128)
            nc.tensor.matmul(psv, pww[:, t * 128:t * 128 + 128],
                             ms[t][:, c0:c0 + cn, :], start=True, stop=True)
            nc.scalar.activation(out=ms[t][:, c0:c0 + cn, :], in_=psv,
                                 func=AF.Copy)
            nc.vector.tensor_scalar_max(
                out=ms[t][:, c0:c0 + cn, :], in0=ms[t][:, c0:c0 + cn, :],
                scalar1=cm[:, t:t + 1])

    def emit_mask_qkv(t, r0, r1):
        nc.vector.tensor_scalar_max(out=qkv[t][:, r0:r1, :],
                                    in0=qkv[t][:, r0:r1, :],
                                    scalar1=cm[:, t:t + 1])

    def emit_kv(r0, r1):
        for r in range(r0, r1):
            tp_ = tps.tile([128, 768], BF16, tag="tp", name="tp")
            for t in range(3):
                nc.tensor.transpose(tp_[:, 128 * t:128 * t + 128],
                                    qkv[t][:, 3 + r, 2:130], idn)
                nc.tensor.transpose(tp_[:, 384 + 128 * t:384 + 128 * t + 128],
                                    ms[t][:, 1 + r, :], idn)
            mT = mtp.tile([128, 768], BF16, tag="mT", name="mT")
            # Pool/GPSIMD cannot read PSUM; DVE is saturated until the last
            # kv rows, so split the psum evacuation Act/DVE by row
            if r < 48:
                nc.scalar.activation(out=mT, in_=tp_, func=AF.Copy)
            else:
                nc.vector.tensor_copy(out=mT, in_=tp_)
            mg = mT.rearrange("p (g c) -> p g c", c=24)
            # matmul stationary operand needs a single free dim: gather k cols
            kc = mtp.tile([128, 256], BF16, tag="kc", name="kc")
            nc.gpsimd.tensor_copy(out=kc.rearrange("p (g e) -> p g e", e=8),
                                  in_=mg[:, :, 8:16])
            for h in range(2):
                kcols = kc[:, 128 * h:128 * h + 128]
                vcols = mg[:, 16 * h:16 * h + 16, 16:24]
                nc.tensor.matmul(kvps[:, 128 * h:128 * h + 128], kcols, vcols,
                                 start=(r == 0), stop=(r == 63),
                                 skip_group_check=True)
                nc.tensor.matmul(kvps[:, 256 + h:257 + h], kcols, ones,
                                 start=(r == 0), stop=(r == 63),
                                 skip_group_check=True)

    # ---- interleaved emission (PE stream must never block on DVE work
    # that gates only LATER PE items) ----
    for k in range(3):                       # conv rows 3..50 (12 chunks)
        for t in (2, 1, 0):
            emit_conv(t, conv_chunks[4 * k:4 * k + 4])
    emit_dwf(0, dw_chunks[0:6])
    emit_dwf(1, dw_chunks[0:6])
    emit_dw5_dve(2, 0, 22)
    emit_dwf(0, dw_chunks[6:11])
    emit_dw5_dve(2, 22, 44)
    emit_dw5_dve(1, 24, 44)
    for k in range(3, 5):                    # conv rows 51..69
        for t in (2, 1, 0):
            emit_conv(t, conv_chunks[4 * k:4 * k + 4])
    emit_pw(2, dw_chunks[0:11])
    emit_dwf(0, dw_chunks[11:17])
    emit_pw(1, dw_chunks[6:11])
    for t in range(3):
        emit_mask_qkv(t, 0, 36)
    emit_kv(0, 33)                           # kv row r needs qkv row 3+r, ms 1+r
    emit_dw5_dve(1, 44, 52)
    emit_dw5_dve(2, 44, 52)
    emit_dwf(2, dw_chunks[13:17])
    emit_dwf(1, dw_chunks[13:17])
    emit_pw(1, dw_chunks[11:13])
    emit_pw(2, dw_chunks[11:13])
    for t in range(3):
        emit_mask_qkv(t, 36, 70)
    emit_kv(33, 64)

    # comp extract: [128, 288] = 2 h-blocks of (16 g x 9): col 144h + 9*gl + dd
    comp = env["comp"]
    compv = comp.rearrange("p (h g e) -> p h g e", h=2, e=9)
    kvv = kvps[:, 0:256].rearrange("p (h g e) -> p h g e", h=2, e=8)
    for h in range(2):
        nc.scalar.activation(out=compv[:, h, :, 0:8], in_=kvv[:, h, :, :],
                             func=AF.Copy)
        ksrc = bass.AP(tensor=kvps.tensor, offset=kvps.offset + 256 + h,
                       ap=[[kvps.ap[0][0], 128], [0, 16], [1, 1]])
        nc.scalar.activation(out=compv[:, h, :, 8:9], in_=ksrc, func=AF.Copy)


def _mid(nc, tc, d, env, ctx):
    """AllReduce + scatter + fused att9 + proj + residual -> attf."""
    qkv, ms, attf = env["qkv"], env["ms"], env["attf"]
    pjw, pjb = env["pjw"], env["pjb"]
    comp, compR, wp = env["comp"], env["compR"], env["wp"]
    cc_in, cc_out, rscr = env["cc_in"], env["cc_out"], env["rscr"]

    if DBG:
        nc.sync.dma_start(out=d["dcomp"], in_=comp)
    nc.sync.dma_start(out=cc_in[:], in_=comp)
    nc.gpsimd.collective_compute(
        "AllReduce", Alu.add,
        replica_groups=[[0, 1], [2, 3], [4, 5], [6, 7]],
        ins=[cc_in.opt()], outs=[cc_out.opt()])
    nc.sync.dma_start(out=compR, in_=cc_out[:])
    if DBG:
        nc.sync.dma_start(out=d["dcompR"], in_=compR)

    lsp = ctx.enter_context(tc.tile_pool(name="lsp", bufs=1))
    lhsT_att = {}
    for a in ATT_SRCS:
        for S in ATT_SRCS[a]:
            st = lsp.tile([128, 108], F32, tag=f"lst{a}_{S}", name=f"lst{a}_{S}")
            nc.gpsimd.memset(st, 0.0)
            bt = lsp.tile([128, 108], BF16, tag=f"lat{a}_{S}", name=f"lat{a}_{S}")
            lhsT_att[(a, S)] = [st, bt]
    for g in range(32):
        a, gl9 = g // 12, g % 12
        h, gl = g // 16, g % 16
        S, row0 = q_chan(g, 0) // 128, q_chan(g, 0) % 128
        st = lhsT_att[(a, S)][0]
        L = st.rearrange("p (dd gl) -> p dd gl", gl=12)
        nc.sync.dma_start(
            out=L[row0:row0 + 8, 0:9, gl9:gl9 + 1],
            in_=compR[8 * gl:8 * gl + 8, 144 * h + 9 * gl:144 * h + 9 * gl + 9])
    for key, (st, bt) in lhsT_att.items():
        nc.vector.tensor_copy(out=bt, in_=st)
        lhsT_att[key] = bt

    eb = env["wtile"]("eb", [12, 96], BF16)

    aps = ctx.enter_context(tc.tile_pool(name="aps", bufs=6, space="PSUM"))
    jps = ctx.enter_context(tc.tile_pool(name="jps", bufs=1, space="PSUM"))
    dxp = ctx.enter_context(tc.tile_pool(name="dxp", bufs=1, space="PSUM"))
    dnp = ctx.enter_context(tc.tile_pool(name="dnp", bufs=3))
    arp = ctx.enter_context(tc.tile_pool(name="arp", bufs=4))

    mchunks = _chunks(0, 66, 4)
    psl_all = {}

    def emit_att9(ci):
        c0, cn = mchunks[ci]
        cw = cn * 128

        def att_rhs(S):
            if S < 3:
                return qkv[S][:, 2 + c0:2 + c0 + cn, 2:130]
            return ms[S - 3][:, c0:c0 + cn, :]

        psl = []
        for a in range(3):
            ps = aps.tile([108, 512], F32, tag="aps", name="aps")
            srcs = ATT_SRCS[a]
            for i, S in enumerate(srcs):
                nc.tensor.matmul(ps[:, 0:cw], lhsT_att[(a, S)], att_rhs(S),
                                 start=(i == 0), stop=(i == len(srcs) - 1))
            psl.append(ps)
        psl_all[ci] = psl

    def emit_div(ci):
        c0, cn = mchunks[ci]
        cw = cn * 128
        psl = psl_all.pop(ci)
        den = dnp.tile([12, 3, 512], F32, tag="den", name="den")
        for a in range(3):
            # +eps so padding groups (den == 0) divide to 0, not NaN
            nc.scalar.activation(out=den[0:12, a, 0:cw],
                                 in_=psl[a][96:108, 0:cw], func=AF.Identity,
                                 bias=env["epsv"][0:12, 0:1], scale=1.0)
        rec = dnp.tile([12, 3, 512], BF16, tag="rec", name="rec")
        with nc.allow_low_precision(reason="den recip to bf16"):
            nc.vector.reciprocal(out=rec[:, :, 0:cw], in_=den[:, :, 0:cw])
        jp = jps.tile([128, 512], F32, tag="jps", name="jps")
        dexp = dnp.tile([96, 3, 512], BF16, tag="dexp", name="dexp")
        for a in range(3):
            # broadcast 12 group-divisors to 96 (dd,gl) rows via 0/1 matmul
            dxt = dxp.tile([96, 512], F32, tag="dxt", name="dxt")
            nc.tensor.matmul(dxt[:, 0:cw], eb[0:12, 0:96],
                             rec[:, a, 0:cw], start=True, stop=True)
            nc.scalar.activation(out=dexp[:, a, 0:cw], in_=dxt[:, 0:cw],
                                 func=AF.Copy)
            attc = arp.tile([96, 512], BF16, tag="attc", name="attc")
            nc.vector.tensor_mul(out=attc[:, 0:cw], in0=dexp[:, a, 0:cw],
                                 in1=psl[a][0:96, 0:cw])
            nc.tensor.matmul(jp[:, 0:cw], pjw[0:96, a * 128:a * 128 + 128],
                             attc[:, 0:cw], start=(a == 0), stop=(a == 2))
        attB = arp.tile([128, 512], BF16, tag="attB", name="attB")
        nc.scalar.activation(out=attB[:, 0:cw], in_=jp[:, 0:cw],
                             func=AF.Identity, bias=pjb[:, 0:1], scale=1.0)
        rbt = arp.tile([128, 4, 128], BF16, tag="rbt", name="rbt")
        nc.scalar.dma_start(out=rbt[:, 0:cn, :], in_=d["rb"][:, c0:c0 + cn, :])
        nc.gpsimd.tensor_add(
            out=attf[:, c0 * 128:c0 * 128 + cw], in0=attB[:, 0:cw],
            in1=rbt[:, 0:cn, :].rearrange("p r w -> p (r w)"))

    emit_att9(0)
    for ci in range(1, 17):
        emit_att9(ci)
        emit_div(ci - 1)
    emit_div(16)


def _tail(nc, tc, d, env, ctx):
    """mb1 + hswish -> h1 (SBUF); dw3 (PE/DVE/Pool) + hswish + mb3 + out."""
    attf = env["attf"]
    if DBG:
        nc.sync.dma_start(out=d["dattf"], in_=attf)
    m1w, m1b, h1b = env["m1w"], env["m1b"], env["h1b"]
    dw3, dw3d, m2b, h2b = env["dw3"], env["dw3d"], env["m2b"], env["h2b"]
    m3w, m3b = env["m3w"], env["m3b"]
    h1 = env["h1"]

    # shared scratch psum: mb1 and dw3-PE outputs round-robin one tag
    sps = ctx.enter_context(tc.tile_pool(name="sps", bufs=3, space="PSUM"))
    m3ps = ctx.enter_context(tc.tile_pool(name="m3ps", bufs=1, space="PSUM"))
    hwp = ctx.enter_context(tc.tile_pool(name="hwp", bufs=4))
    mulp = ctx.enter_context(tc.tile_pool(name="mulp", bufs=7))
    thp = ctx.enter_context(tc.tile_pool(name="thp", bufs=4))
    xap = ctx.enter_context(tc.tile_pool(name="xap", bufs=3))
    accp = ctx.enter_context(tc.tile_pool(name="accp", bufs=3))
    osp = ctx.enter_context(tc.tile_pool(name="osp", bufs=2))

    for t in range(6):
        nc.gpsimd.memset(h1[t][:, :, 0:1], 0.0)
        nc.gpsimd.memset(h1[t][:, :, 129:130], 0.0)
        nc.gpsimd.memset(h1[t][:, 0:1, :], 0.0)

    p8_chunks = _chunks(0, 66, 8)   # 9
    p9_chunks = _chunks(0, 64, 8)   # 8

    def emit_p8(j):
        c0, cn = p8_chunks[j]
        for t in range(6):
            ps = sps.tile([128, 1024], F32, tag="sps", name="sps")
            for (s0, sn) in _chunks(c0, c0 + cn, 4):
                nc.tensor.matmul(
                    ps[:, (s0 - c0) * 128:(s0 - c0 + sn) * 128],
                    m1w[:, t * 128:t * 128 + 128],
                    attf[:, s0 * 128:(s0 + sn) * 128],
                    start=True, stop=True)
            pw_ = ps[:, 0:cn * 128]
            th = hwp.tile([128, 1024], BF16, tag="th", name="th")
            nc.scalar.activation(out=th[:, 0:cn * 128], in_=pw_, func=AF.Relu,
                                 bias=h1b[:, t:t + 1], scale=1.0 / 6.0)
            nc.vector.tensor_scalar_min(out=th[:, 0:cn * 128],
                                        in0=th[:, 0:cn * 128], scalar1=1.0)
            r0 = max(c0, 1)   # keep h1 row 0 zero (dw3 top pad)
            off = (r0 - c0) * 128
            hout = h1[t][:, r0:c0 + cn, 1:129]
            thv = th[:, off:cn * 128].rearrange("p (r w) -> p r w", w=128)
            if t < 4:
                # xh on Act, multiply on Pool (SBUF-only engine)
                xh = hwp.tile([128, 1024], BF16, tag="xh", name="xh")
                nc.scalar.activation(out=xh[:, 0:cn * 128], in_=pw_,
                                     func=AF.Identity, bias=m1b[:, t:t + 1],
                                     scale=1.0)
                nc.gpsimd.tensor_mul(
                    out=hout, in0=thv,
                    in1=xh[:, off:cn * 128].rearrange("p (r w) -> p r w",
                                                      w=128))
            else:
                # fused (psum + m1b) * min(relu(.),1) on DVE
                pv = ps[:, off:cn * 128].rearrange("p (r w) -> p r w", w=128)
                nc.vector.scalar_tensor_tensor(
                    out=hout, in0=pv, scalar=m1b[:, t:t + 1], in1=thv,
                    op0=Alu.add, op1=Alu.mult)

    h2fs_all = {}

    def emit_p9p1(j):
        q0, qn = p9_chunks[j]
        h2fs = [None] * 6
        # pass 1: dw3 + hswish per tile (PE runs ahead across tiles); PE
        # tiles first so their hswish (DVE) unblocks mb3 early, DVE-dw3 last
        for t in (0, 1, 2, 3, 5, 4):
            eng = DW3_ENG[t]
            h2 = mulp.tile([128, 8, 128], BF16, tag="h2", name="h2")
            if eng == "pe":
                ps = sps.tile([128, 1024], F32, tag="sps", name="sps")
                psv = ps[:, 0:qn * 128].rearrange("p (r w) -> p r w", w=128)
                for (s0, sn) in _chunks(0, qn, 4):
                    for tap in range(9):
                        dy, dx = tap // 3, tap % 3
                        k = (t * 9 + tap) * 128
                        nc.tensor.matmul(
                            psv[:, s0:s0 + sn, :], dw3d[:, k:k + 128],
                            h1[t][:, q0 + s0 + dy:q0 + s0 + dy + sn,
                                  dx:dx + 128],
                            start=(tap == 0), stop=(tap == 8))
                th2 = thp.tile([128, 1024], BF16, tag="th2", name="th2")
                nc.scalar.activation(out=th2[:, 0:qn * 128],
                                     in_=ps[:, 0:qn * 128], func=AF.Relu,
                                     bias=h2b[:, t:t + 1], scale=1.0 / 6.0)
                if t < 2:
                    xa = xap.tile([128, 1024], BF16, tag="xa", name="xa")
                    nc.scalar.activation(out=xa[:, 0:qn * 128],
                                         in_=ps[:, 0:qn * 128],
                                         func=AF.Identity,
                                         bias=m2b[:, t:t + 1], scale=1.0)
                    accf = xa[:, 0:qn * 128]
                else:
                    accf = ps[:, 0:qn * 128]   # fused add in STT below
            else:
                acc = accp.tile([128, 8, 128], BF16, tag="acc", name="acc")
                av = acc[:, 0:qn, :]
                for tap in range(9):
                    dy, dx = tap // 3, tap % 3
                    w_ap = dw3[:, t * 9 + tap:t * 9 + tap + 1]
                    win = h1[t][:, q0 + dy:q0 + dy + qn, dx:dx + 128]
                    if eng == "dve":
                        if tap == 0:
                            nc.vector.tensor_scalar(
                                out=av, in0=win, scalar1=w_ap,
                                scalar2=m2b[:, t:t + 1], op0=Alu.mult,
                                op1=Alu.add)
                        else:
                            tmp = accp.tile([128, 8, 128], BF16, tag="tmp",
                                           name="tmp")
                            nc.vector.tensor_scalar_mul(
                                out=tmp[:, 0:qn, :], in0=win, scalar1=w_ap)
                            nc.vector.tensor_add(out=av, in0=av,
                                                 in1=tmp[:, 0:qn, :])
                    else:
                        if tap == 0:
                            nc.gpsimd.tensor_scalar(
                                out=av, in0=win, scalar1=w_ap,
                                scalar2=m2b[:, t:t + 1], op0=Alu.mult,
                                op1=Alu.add)
                        else:
                            nc.gpsimd.scalar_tensor_tensor(
                                out=av, in0=win, scalar=w_ap, in1=av,
                                op0=Alu.mult, op1=Alu.add)
                th2 = thp.tile([128, 1024], BF16, tag="th2", name="th2")
                accf = av.rearrange("p r w -> p (r w)")
                nc.scalar.activation(out=th2[:, 0:qn * 128], in_=accf,
                                     func=AF.Relu, bias=env["half"][:, 0:1],
                                     scale=1.0 / 6.0)
            nc.vector.tensor_scalar_min(out=th2[:, 0:qn * 128],
                                        in0=th2[:, 0:qn * 128], scalar1=1.0)
            h2f = h2.rearrange("p r w -> p (r w)")[:, 0:qn * 128]
            if eng == "pe" and t >= 2:
                nc.vector.scalar_tensor_tensor(
                    out=h2f, in0=accf, scalar=m2b[:, t:t + 1],
                    in1=th2[:, 0:qn * 128], op0=Alu.add, op1=Alu.mult)
            elif t < 2:
                nc.gpsimd.tensor_mul(out=h2f, in0=th2[:, 0:qn * 128],
                                     in1=accf)
            else:
                nc.vector.tensor_mul(out=h2f, in0=th2[:, 0:qn * 128],
                                     in1=accf)
            h2fs[t] = h2f
        h2fs_all[j] = h2fs

    def emit_p9p2(j):
        q0, qn = p9_chunks[j]
        h2fs = h2fs_all.pop(j)
        mp = m3ps.tile([128, 1024], F32, tag="m3ps", name="m3ps")
        # mb3 accumulation (DVE-dw3 tile last — ready latest)
        p2 = (0, 1, 2, 3, 5, 4)
        for t in p2:
            for (s0, sn) in _chunks(0, qn, 4):
                nc.tensor.matmul(
                    mp[:, s0 * 128:(s0 + sn) * 128],
                    m3w[:, t * 128:t * 128 + 128],
                    h2fs[t][:, s0 * 128:(s0 + sn) * 128],
                    start=(t == p2[0]), stop=(t == p2[-1]))
        o1 = osp.tile([128, 1024], BF16, tag="o1", name="o1")
        nc.scalar.activation(out=o1[:, 0:qn * 128], in_=mp[:, 0:qn * 128],
                             func=AF.Identity, bias=m3b[:, 0:1], scale=1.0)
        nc.vector.tensor_add(
            out=o1[:, 0:qn * 128], in0=o1[:, 0:qn * 128],
            in1=attf[:, (q0 + 1) * 128:(q0 + 1 + qn) * 128])
        nc.sync.dma_start(
            out=d["out"][:, q0:q0 + qn, :],
            in_=o1[:, 0:qn * 128].rearrange("p (r w) -> p r w", w=128))

    emit_p8(0)
    emit_p8(1)
    emit_p9p1(0)
    for j in range(2, 9):
        emit_p8(j)
        emit_p9p2(j - 2)
        emit_p9p1(j - 1)
    emit_p9p2(7)
    if DBG:
        for t in range(6):
            nc.sync.dma_start(out=d["dh1"][t], in_=h1[t])


# ====================== host side ======================

def _prep_shared(inp):
    f32 = np.float32
    out = {}
    pw = inp["agg_pw_w"][:, :, 0, 0]
    s1v = inp["bn1_g"] / np.sqrt(inp["bn1_v"] + BN_EPS)
    b1 = inp["bn1_b"] - inp["bn1_m"] * s1v
    Wp = inp["attn_proj_w"][:, :, 0, 0] * s1v[:, None]
    s2 = inp["bn2_g"] / np.sqrt(inp["bn2_v"] + BN_EPS)
    b2 = inp["bn2_b"] - inp["bn2_m"] * s2
    W3 = inp["mb3_w"][:, :, 0, 0] * s2[:, None]
    idn = np.eye(128, dtype=f32)

    for s in (0, 1):
        w = {}
        wc = np.zeros((128, 27 * 128), f32)
        for j, cw in enumerate((inp["wq"], inp["wk"], inp["wv"])):
            for dy in range(3):
                dyy = 2 - dy if s == 1 else dy
                for dx in range(3):
                    k = (j * 9 + dy * 3 + dx) * 128
                    wc[:, k:k + 128] = cw[:, :, dyy, dx].T
        w["wc"] = wc.astype(BF)
        w["cb"] = np.stack([inp["bq"], inp["bk"], inp["bv"]], 1).astype(f32)
        m = np.arange(384)
        w["cm"] = np.where((m % 24) < 16, 0.0, -1e9).astype(f32).reshape(3, 128).T.copy()
        dw5 = np.zeros((128, 75), f32)
        for t in range(3):
            for tap in range(25):
                dy, dx = tap // 5, tap % 5
                dyy = 4 - dy if s == 1 else dy
                dw5[:, t * 25 + tap] = inp["agg_dw_w"][128 * t:128 * t + 128, 0, dyy, dx]
        w["dw5"] = dw5
        # block-diag pw (per tile): pwbd[t][i, o] nonzero iff i//8 == o//8
        pwbd = np.zeros((3, 128, 128), f32)
        for mc in range(384):
            t, o = mc // 128, mc % 128
            g8 = (o // 8) * 8
            pwbd[t, g8:g8 + 8, o] = pw[mc]
        w["pww"] = pwbd.transpose(1, 0, 2).reshape(128, 384).astype(BF)
        # fused dw5+pw: wf[i, (t*25+tap)*128+o] = dw5[i,tap] * pwbd[t][i,o]
        wf = np.zeros((128, 3 * 25 * 128), f32)
        for t in range(3):
            for tap in range(25):
                k = (t * 25 + tap) * 128
                wf[:, k:k + 128] = pwbd[t] * dw5[:, t * 25 + tap][:, None]
        w["wf"] = wf.astype(BF)
        pjw = np.zeros((128, 3 * 128), f32)
        for g in range(32):
            a, gl9 = g // 12, g % 12
            for dd in range(8):
                pjw[12 * dd + gl9, a * 128:a * 128 + 128] = Wp[:, 8 * g + dd]
        w["pjw"] = pjw.astype(BF)
        w["pjb"] = b1.reshape(128, 1).astype(f32)
        m1w = np.zeros((128, 6 * 128), f32)
        for t in range(6):
            m1w[:, t * 128:t * 128 + 128] = inp["mb1_w"][128 * t:128 * t + 128, :, 0, 0].T
        w["m1w"] = m1w.astype(BF)
        w["m1b"] = inp["mb1_b"].reshape(6, 128).T.copy().astype(f32)
        w["h1b"] = (inp["mb1_b"].reshape(6, 128).T / 6.0 + 0.5).astype(f32)
        dw3 = np.zeros((128, 54), f32)
        for t in range(6):
            for tap in range(9):
                dy, dx = tap // 3, tap % 3
                dyy = 2 - dy if s == 1 else dy
                dw3[:, t * 9 + tap] = inp["mb2_w"][128 * t:128 * t + 128, 0, dyy, dx]
        w["dw3"] = dw3
        dw3d = np.zeros((128, 54 * 128), f32)
        for c in range(54):
            dw3d[:, c * 128:c * 128 + 128] = np.diag(dw3[:, c])
        w["dw3d"] = dw3d.astype(BF)
        w["m2b"] = inp["mb2_b"].reshape(6, 128).T.copy().astype(f32)
        w["h2b"] = (inp["mb2_b"].reshape(6, 128).T / 6.0 + 0.5).astype(f32)
        m3w = np.zeros((128, 6 * 128), f32)
        for t in range(6):
            m3w[:, t * 128:t * 128 + 128] = W3[:, 128 * t:128 * t + 128].T
        w["m3w"] = m3w.astype(BF)
        w["m3b"] = b2.reshape(128, 1).astype(f32)
        w["idn"] = idn.astype(BF)
        eb = np.zeros((12, 96), f32)
        for p in range(96):
            eb[p % 12, p] = 1.0
        w["eb"] = eb.astype(BF)
        out[s] = w
    return out


def _prep_core(inp, b, s):
    f32 = np.float32
    ref = inp["ref_features"][b]
    oth = inp["other_features"][b]
    if s == 1:
        ref = ref[:, ::-1, :]
        oth = oth[:, ::-1, :]
    xr = np.zeros((128, 72, 130), f32)
    xo = np.zeros((128, 72, 130), f32)
    xr[:, 4:72, 1:129] = ref[:, 0:68, :]
    xo[:, 4:72, 1:129] = oth[:, 0:68, :]
    rr = np.zeros((128, 66, 128), f32)
    rr[:, 1:66, :] = ref[:, 0:65, :]
    return {"xr": xr.astype(BF), "xo": xo.astype(BF), "rb": rr.astype(BF)}


def kernel(**inputs):
    inp = {k: np.asarray(v) for k, v in inputs.items()}
    if "nc" not in _CACHE:
        _CACHE["nc"] = build_program()
    nc = _CACHE["nc"]
    ws = _prep_shared(inp)
    in_maps = []
    for c in range(NCORES):
        b, s = c // 2, c % 2
        m = dict(ws[s])
        m.update(_prep_core(inp, b, s))
        in_maps.append(m)
    res = bass_utils.run_bass_kernel_spmd(nc, in_maps,
                                          core_ids=list(range(NCORES)))
    out = np.zeros((4, 128, 128, 128), np.float32)
    for c in range(NCORES):
        b, s = c // 2, c % 2
        o = res.results[c]["out"].astype(np.float32)
        if s == 1:
            o = o[:, ::-1, :]
        out[b, :, 64 * s:64 * s + 64, :] = o
    return out
